# revision 1
# baseline (speedup 1.0000x reference)
"""Multi-head attention (B=2, S=2048, D=1024, H=16) on 8 trn2 NeuronCores.

Sharding: data-parallel over batch (2) x tensor-parallel over head-groups (4).
Core c handles batch c//4, heads [4*(c%4), 4*(c%4)+4).  Each core computes
q/k/v projections for its 256 head-features, masked softmax attention for its
4 heads, and the row-shard of the output projection; partial outputs are
summed on the host during the gather step.
"""

import sys
import functools
from contextlib import ExitStack

sys.path.insert(0, "/opt/trn_rl_repo")

import numpy as np

B, S, D, H = 2, 2048, 1024, 16
DH = 64
P = 128
NCORES = 8
GROUPS = 4            # head groups = cores per batch
NH = H // GROUPS      # heads per core = 4
F = NH * DH           # head features per core = 256
KS = S // P           # 16 key slices
QT = S // P           # 16 query tiles
DS = D // P           # 8 d_model slices
VW = DH + 1           # v width per head incl. ones column = 65


def _emit(nc, tc, t, dbg=None):
    import concourse.mybir as mybir
    bf16 = mybir.dt.bfloat16
    f32 = mybir.dt.float32
    Exp = mybir.ActivationFunctionType.Exp
    Copy = mybir.ActivationFunctionType.Copy

    es = ExitStack()
    const_pool = es.enter_context(tc.tile_pool(name="const", bufs=1))
    w_pool = es.enter_context(tc.tile_pool(name="w", bufs=1))
    x_pool = es.enter_context(tc.tile_pool(name="x", bufs=2))
    qk_pool = es.enter_context(tc.tile_pool(name="qk", bufs=1))
    v_pool = es.enter_context(tc.tile_pool(name="v", bufs=1))
    nm_pool = es.enter_context(tc.tile_pool(name="nm", bufs=3))
    pt_pool = es.enter_context(tc.tile_pool(name="pt", bufs=7))
    ctxT_pool = es.enter_context(tc.tile_pool(name="ctxT", bufs=1))
    rs_pool = es.enter_context(tc.tile_pool(name="rs", bufs=4))
    ps_pool = es.enter_context(tc.tile_pool(name="ps", bufs=2, space="PSUM"))
    psp_cm = tc.tile_pool(name="psproj", bufs=4, space="PSUM")
    psp_pool = psp_cm.__enter__()

    bq_sb = const_pool.tile([P, 2], f32, tag="bq")
    bk_sb = const_pool.tile([P, 2], f32, tag="bk")

    # Weights: [128, DS*F] layout, col = ds*F + f
    wq_sb = w_pool.tile([P, DS * F], bf16, tag="wq")
    wk_sb = w_pool.tile([P, DS * F], bf16, tag="wk")
    wv_sb = w_pool.tile([P, DS * F], bf16, tag="wv")
    wom_sb = w_pool.tile([P, 2 * D], bf16, tag="wom")
    def load_w(w_sb, wname, ng):
        nc.sync.dma_start(
            w_sb[:].rearrange("p (g f) -> p g f", g=ng),
            t[wname].ap().rearrange("(g p) f -> p g f", p=P),
        )

    load_w(wq_sb, "wqT", DS)
    nc.sync.dma_start(bq_sb[:], t["bq"].ap().rearrange("(ft p) one -> p (ft one)", p=P))
    nc.sync.dma_start(bk_sb[:], t["bk"].ap().rearrange("(ft p) one -> p (ft one)", p=P))

    # ---- Phase A: projections ----
    # qT/kT [256, S] bf16 as 2 tiles of [128, S] (partition = head-feature)
    qT = [qk_pool.tile([P, S], bf16, tag=f"qT{ft}", name=f"qT{ft}") for ft in range(2)]
    kT = [qk_pool.tile([P, S], bf16, tag=f"kT{ft}", name=f"kT{ft}") for ft in range(2)]
    # v [token, feat] with per-head ones column: [128, KS * NH * VW]
    v_sb = v_pool.tile([P, KS * NH * VW], bf16, tag="v")
    nc.gpsimd.memset(v_sb[:], 1.0)

    def load_x(x_name):
        x_sb = x_pool.tile([P, DS * S], bf16, tag="xfull", name=f"x_{x_name}")
        # tc2-major chunking: the first accumulation groups only need the
        # first 1024-column chunk of every d-slice, so they can start after
        # ~2MB of DMA instead of the full 4MB tensor.
        for tc2 in range(2):
            for dh in range(2):
                nc.sync.dma_start(
                    x_sb[:].rearrange("p (ds c) -> p ds c", ds=DS)[:, dh * 4:(dh + 1) * 4, tc2 * 1024:(tc2 + 1) * 1024],
                    t[x_name].ap().rearrange("(ds p) c -> p ds c", p=P)[:, dh * 4:(dh + 1) * 4, tc2 * 1024:(tc2 + 1) * 1024],
                )
        return x_sb

    def qk_proj_ft(x_sb, w_sb, b_sb, out_tiles, ft):
        for tc4 in range(4):
            ps = psp_pool.tile([P, 512], f32, tag="psproj")
            for ds in range(DS):
                nc.tensor.matmul(
                    ps[:],
                    w_sb[:, ds * F + ft * P: ds * F + (ft + 1) * P],
                    x_sb[:, ds * S + tc4 * 512: ds * S + (tc4 + 1) * 512],
                    start=(ds == 0),
                    stop=(ds == DS - 1),
                )
            nc.vector.tensor_scalar_add(
                out_tiles[ft][:, tc4 * 512:(tc4 + 1) * 512], ps[:], b_sb[:, ft:ft + 1]
            )

    xq_sb = load_x("xqT")
    qk_proj_ft(xq_sb, wq_sb, bq_sb, qT, 0)
    load_w(wv_sb, "wvT", DS)
    qk_proj_ft(xq_sb, wq_sb, bq_sb, qT, 1)
    load_w(wk_sb, "wkT", DS)
    load_w(wom_sb, "womT", 2)

    # v projection before k: the attention stream (scores -> exp) starts the
    # moment kT is done, with no v-projection bubble on the ACT engine.
    xv_sb = x_pool.tile([P, DS * S], bf16, tag="xfull", name="x_xvT")
    for tc2 in range(2):
        for dh in range(2):
            nc.sync.dma_start(
                xv_sb[:].rearrange("p (ds c) -> p ds c", ds=DS)[:, dh * 4:(dh + 1) * 4, tc2 * 1024:(tc2 + 1) * 1024],
                t["xvT"].ap().rearrange("(ds p) c -> p ds c", p=P)[:, dh * 4:(dh + 1) * 4, tc2 * 1024:(tc2 + 1) * 1024],
            )
    for tt in range(KS):
        ps = psp_pool.tile([P, F], f32, tag="psproj")
        for ds in range(DS):
            nc.tensor.matmul(
                ps[:],
                xv_sb[:, ds * S + tt * P: ds * S + tt * P + P],
                wv_sb[:, ds * F:(ds + 1) * F],
                start=(ds == 0),
                stop=(ds == DS - 1),
            )
        dst = v_sb[:, tt * NH * VW:(tt + 1) * NH * VW].rearrange("p (h w) -> p h w", h=NH)[:, :, 0:DH]
        nc.vector.tensor_copy(dst, ps[:].rearrange("p (h f) -> p h f", h=NH))
    if dbg is not None:
        nc.sync.dma_start(dbg["v"].ap(), v_sb[:])

    xk_sb = load_x("xkT")
    qk_proj_ft(xk_sb, wk_sb, bk_sb, kT, 0)
    qk_proj_ft(xk_sb, wk_sb, bk_sb, kT, 1)
    if dbg is not None:
        for ft in range(2):
            nc.sync.dma_start(dbg["qT"].ap()[ft * P:(ft + 1) * P, :], qT[ft][:])
            nc.sync.dma_start(dbg["kT"].ap()[ft * P:(ft + 1) * P, :], kT[ft][:])

    psp_cm.__exit__(None, None, None)

    # ---- Phase B setup (scores closures; ctx^T accumulation) ----
    # ctx accumulated transposed: ctx^T[f, q] = sum_k v_ext[k, f] * pT[k, q]
    # (stationary = v slice, moving = pT strips -> few wide matmuls instead of
    # many narrow ones; the PE sequencer dispatch rate is the limit otherwise).
    # Row DH of ctx^T is the softmax denominator via the ones column of v_ext.
    ctxT = [ctxT_pool.tile([P, S], bf16, tag=f"ctxT{h}", name=f"ctxT{h}") for h in range(2)]
    ctx_cm = tc.tile_pool(name="ctxps", bufs=1, space="PSUM")
    ctx_pool = ctx_cm.__enter__()
    all_pts = {h: [None] * KS for h in range(NH)}

    def scores(h, ks):
        ft, ro = h // 2, (h % 2) * DH
        pt = pt_pool.tile([P, S], bf16, tag="pt")
        nm = nm_pool.tile([P, S], bf16, tag="nm")
        nc.gpsimd.dma_start(nm[:], t["nmT"].ap()[ks * P:(ks + 1) * P, :])
        for qc in range(2):
            ps = ps_pool.tile([P, 1024], f32, tag="ps")
            for qh in range(2):
                nc.tensor.matmul(
                    ps[:, qh * 512:(qh + 1) * 512],
                    kT[ft][ro:ro + DH, ks * P:(ks + 1) * P],
                    qT[ft][ro:ro + DH, qc * 1024 + qh * 512: qc * 1024 + (qh + 1) * 512],
                    start=True,
                    stop=True,
                )
            nc.scalar.activation(pt[:, qc * 1024:(qc + 1) * 1024], ps[:], Exp, scale=0.125)
            nc.vector.tensor_mul(
                pt[:, qc * 1024:(qc + 1) * 1024],
                pt[:, qc * 1024:(qc + 1) * 1024],
                nm[:, qc * 1024:(qc + 1) * 1024],
            )
        all_pts[h][ks] = pt
        if dbg is not None and h == 0:
            nc.sync.dma_start(dbg["pT0"].ap()[ks * P:(ks + 1) * P, :], pt[:])

    EARLY = 0

    # ---- Phase B: attention per head, software-pipelined over key slices ----
    DEPTH = 5  # software-pipeline offset between scores and ctx
    for h in range(NH):
        ft, ro = h // 2, (h % 2) * DH
        ctx_ps = ctx_pool.tile([P, S], f32, tag="ctx")
        pts = all_pts[h]

        def ctx_step(ks):
            pt = pts[ks]
            for qc4 in range(4):
                nc.tensor.matmul(
                    ctx_ps[0:VW, qc4 * 512:(qc4 + 1) * 512],
                    v_sb[:, ks * NH * VW + h * VW: ks * NH * VW + (h + 1) * VW],
                    pt[:, qc4 * 512:(qc4 + 1) * 512],
                    start=(ks == 0),
                    stop=(ks == KS - 1),
                )

        for ks in range(KS + DEPTH):
            if ks < KS and not (h == 0 and ks < EARLY):
                scores(h, ks)
            if ks >= DEPTH:
                ctx_step(ks - DEPTH)

        # normalize: ctxT[f, q] = ctx^T[f, q] / ctx^T[DH, q], in two column
        # chunks so the output projection can start on the first half.
        for nh2 in range(2):
            cs = slice(nh2 * (S // 2), (nh2 + 1) * (S // 2))
            rs = rs_pool.tile([1, S // 2], f32, tag="rs")
            nc.vector.reciprocal(rs[:], ctx_ps[DH:DH + 1, cs])
            rsb = rs_pool.tile([DH, S // 2], f32, tag="rsb")
            nc.gpsimd.partition_broadcast(rsb[:], rs[:])
            nc.vector.tensor_mul(ctxT[ft][ro:ro + DH, cs], ctx_ps[0:DH, cs], rsb[:])

    if dbg is not None:
        for half in range(2):
            nc.sync.dma_start(dbg["ctxT"].ap()[half * P:(half + 1) * P, :], ctxT[half][:])
    ctx_cm.__exit__(None, None, None)

    # ---- Phase D: output projection outT[o, q] = sum_f womT[f, o] * ctxT[f, q] ----
    # opj tiles come from ps_pool: its slots free up right after the last
    # scores' exp, so the first outproj matmuls don't wait on the ctx-pool
    # banks (which are only released once head 3's norm completes).
    out_cm = tc.tile_pool(name="out", bufs=4)
    out_pool = out_cm.__enter__()
    for qc in range(2):
        for ot in range(8):
            ps = ps_pool.tile([P, 1024], f32, tag="ps")
            for fs in range(2):
                for qh in range(2):
                    nc.tensor.matmul(
                        ps[:, qh * 512:(qh + 1) * 512],
                        wom_sb[:, fs * D + ot * P: fs * D + (ot + 1) * P],
                        ctxT[fs][:, qc * 1024 + qh * 512: qc * 1024 + (qh + 1) * 512],
                        start=(fs == 0),
                        stop=(fs == 1),
                    )
            osb = out_pool.tile([P, 1024], f32, tag="osb")
            if (ot * 2 + qc) % 2 == 0:
                nc.vector.tensor_copy(osb[:], ps[:])
            else:
                nc.scalar.copy(osb[:], ps[:])
            for qh in range(2):
                nc.sync.dma_start(
                    t["outT"].ap()[ot * P:(ot + 1) * P, qc * 1024 + qh * 512: qc * 1024 + (qh + 1) * 512],
                    osb[:, qh * 512:(qh + 1) * 512],
                )
    out_cm.__exit__(None, None, None)
    es.close()


@functools.lru_cache(maxsize=1)
def _build(debug=False):
    import concourse.bacc as bacc
    import concourse.mybir as mybir
    import concourse.tile as tile

    bf16 = mybir.dt.bfloat16
    f32 = mybir.dt.float32

    nc = bacc.Bacc("TRN2", target_bir_lowering=False, debug=False, num_devices=NCORES)
    t = {
        "xqT": nc.dram_tensor("xqT", (D, S), bf16, kind="ExternalInput"),
        "xkT": nc.dram_tensor("xkT", (D, S), bf16, kind="ExternalInput"),
        "xvT": nc.dram_tensor("xvT", (D, S), bf16, kind="ExternalInput"),
        "wqT": nc.dram_tensor("wqT", (D, F), bf16, kind="ExternalInput"),
        "wkT": nc.dram_tensor("wkT", (D, F), bf16, kind="ExternalInput"),
        "wvT": nc.dram_tensor("wvT", (D, F), bf16, kind="ExternalInput"),
        "womT": nc.dram_tensor("womT", (F, D), bf16, kind="ExternalInput"),
        "nmT": nc.dram_tensor("nmT", (S, S), bf16, kind="ExternalInput"),
        "bq": nc.dram_tensor("bq", (F, 1), f32, kind="ExternalInput"),
        "bk": nc.dram_tensor("bk", (F, 1), f32, kind="ExternalInput"),
        "outT": nc.dram_tensor("outT", (D, S), f32, kind="ExternalOutput"),
    }
    dbg = None
    if debug:
        dbg = {
            "qT": nc.dram_tensor("dbg_qT", (F, S), bf16, kind="ExternalOutput"),
            "kT": nc.dram_tensor("dbg_kT", (F, S), bf16, kind="ExternalOutput"),
            "v": nc.dram_tensor("dbg_v", (P, KS * NH * VW), bf16, kind="ExternalOutput"),
            "pT0": nc.dram_tensor("dbg_pT0", (S, S), bf16, kind="ExternalOutput"),
            "ctxT": nc.dram_tensor("dbg_ctxT", (F, S), bf16, kind="ExternalOutput"),
        }
    with tile.TileContext(nc) as tc:
        _emit(nc, tc, t, dbg)
    nc.compile()
    return nc


def _prep_core_inputs(c, Q, K, V, mask, Wq, bq, Wk, bk, Wv, Wo, _cache={}):
    import ml_dtypes

    bf = ml_dtypes.bfloat16
    b, g = divmod(c, GROUPS)
    bkey = ("batch", b, id(Q))
    if bkey not in _cache:
        _cache.clear()
        for bb in range(B):
            nm = 1.0 - mask[bb, 0].astype(np.float32)
            _cache[("batch", bb, id(Q))] = {
                "xqT": Q[bb].T.astype(bf),
                "xkT": K[bb].T.astype(bf),
                "xvT": V[bb].T.astype(bf),
                "nmT": nm.T.astype(bf),
            }
    fsl = slice(g * F, (g + 1) * F)
    return {
        **_cache[bkey],
        "wqT": Wq[fsl, :].T.astype(bf),
        "wkT": Wk[fsl, :].T.astype(bf),
        "wvT": Wv[fsl, :].T.astype(bf),
        "womT": Wo[:, fsl].T.astype(bf),
        "bq": bq[fsl].reshape(F, 1).astype(np.float32),
        "bk": bk[fsl].reshape(F, 1).astype(np.float32),
    }


def kernel(Q, K, V, mask, Wq, bq, Wk, bk, Wv, bv, Wo, bo, _trace=False, _tmpdir=None):
    from concourse.bass_utils import run_bass_kernel_spmd

    Q, K, V = np.asarray(Q, np.float32), np.asarray(K, np.float32), np.asarray(V, np.float32)
    mask = np.asarray(mask)
    Wq, Wk, Wv, Wo = (np.asarray(w, np.float32) for w in (Wq, Wk, Wv, Wo))
    bq, bk, bv, bo = (np.asarray(x, np.float32) for x in (bq, bk, bv, bo))

    nc = _build()
    in_maps = [_prep_core_inputs(c, Q, K, V, mask, Wq, bq, Wk, bk, Wv, Wo) for c in range(NCORES)]
    kw = {}
    if _trace:
        kw = dict(trace=True, tmpdir=_tmpdir)
    res = run_bass_kernel_spmd(nc, in_maps, core_ids=list(range(NCORES)), **kw)

    const = (Wo @ bv + bo).astype(np.float32)  # softmax rows sum to 1 -> bv enters linearly
    out = np.empty((B, S, D), np.float32)
    for b in range(B):
        acc = res.results[b * GROUPS]["outT"].astype(np.float32)
        for g in range(1, GROUPS):
            acc = acc + res.results[b * GROUPS + g]["outT"]
        out[b] = acc.T + const
    if _trace:
        kernel._last_results = res
    return out



# revision 12
# speedup vs baseline: 1.0606x; 1.0606x over previous
"""Multi-head attention (B=2, S=2048, D=1024, H=16) on 8 trn2 NeuronCores.

Sharding: data-parallel over batch (2) x tensor-parallel over head-groups (4).
Core c handles batch c//4, heads [4*(c%4), 4*(c%4)+4).  Each core computes
q/k/v projections for its 256 head-features, masked softmax attention for its
4 heads, and the row-shard of the output projection; partial outputs are
summed on the host during the gather step.

Schedule: attention is blocked into 4 query-quarters of 512.  The ctx matmul
runs with pt as the stationary operand (q on the output partitions, head
features on the free dim), which halves its PE cost versus the v-stationary
orientation; the softmax denominator rides along as a 65th v column and is
divided out with a per-partition reciprocal before a PE transpose puts ctx
back into [feature, q] layout for the output projection.  Projections for
later head groups are woven between attention score pairs so the activation
engine (exp) starts early and stays fed.
"""

import sys
import functools
from contextlib import ExitStack

sys.path.insert(0, "/opt/trn_rl_repo")

import numpy as np

B, S, D, H = 2, 2048, 1024, 16
DH = 64
P = 128
NCORES = 8
GROUPS = 4            # head groups = cores per batch
NH = H // GROUPS      # heads per core = 4
F = NH * DH           # head features per core = 256
KS = S // P           # 16 key slices
DS = D // P           # 8 d_model slices
QW = 512              # query quarter width
NQ = S // QW          # 4 quarters
NPAIR = KS // 2       # 8 key-slice pairs
VW = DH + 1           # v width per head incl. ones column = 65
DEPTH = 4             # ctx lag behind scores, in key-slice pairs


def _emit(nc, tc, t, dbg=None):
    import concourse.mybir as mybir
    bf16 = mybir.dt.bfloat16
    f32 = mybir.dt.float32
    Exp = mybir.ActivationFunctionType.Exp

    es = ExitStack()
    ep = lambda cm: es.enter_context(cm)
    const_pool = ep(tc.tile_pool(name="const", bufs=1))
    w_pool = ep(tc.tile_pool(name="w", bufs=1))
    x_pool = ep(tc.tile_pool(name="x", bufs=1))
    qk_pool = ep(tc.tile_pool(name="qk", bufs=1))
    v_pool = ep(tc.tile_pool(name="v", bufs=1))
    nm_pool = ep(tc.tile_pool(name="nm", bufs=2))
    pt_pool = ep(tc.tile_pool(name="pt", bufs=6))
    cn_pool = ep(tc.tile_pool(name="cn", bufs=4))
    rs_pool = ep(tc.tile_pool(name="rs", bufs=4))
    ct_pool = ep(tc.tile_pool(name="ct", bufs=4))
    ob_pool = ep(tc.tile_pool(name="ob", bufs=4))
    # PSUM: pj 2 banks + sc 4 + cx 1 + tr 1 = 8; op (2) reuses pj's banks
    # after the projection pool closes.  pj is opened last so its mid-emit
    # release keeps the pool stack LIFO.
    sc_pool = ep(tc.tile_pool(name="sc", bufs=2, space="PSUM"))
    cx_pool = ep(tc.tile_pool(name="cx", bufs=1, space="PSUM"))
    tr_pool = ep(tc.tile_pool(name="tr", bufs=1, space="PSUM"))
    pj_cm = tc.tile_pool(name="pj", bufs=2, space="PSUM")
    pj_pool = pj_cm.__enter__()

    # ---- SBUF tiles ----
    bq_sb = const_pool.tile([P, 2], f32, tag="bq")
    bk_sb = const_pool.tile([P, 2], f32, tag="bk")
    ident_sb = const_pool.tile([P, P], bf16, tag="ident")
    wu_sb = const_pool.tile([P, QW], bf16, tag="wu")  # warmup garbage

    wq_sb = w_pool.tile([P, DS * F], bf16, tag="wq")
    wk_sb = w_pool.tile([P, DS * F], bf16, tag="wk")
    wv_sb = w_pool.tile([P, DS * F], bf16, tag="wv")
    wom_sb = w_pool.tile([P, 2 * D], bf16, tag="wom")

    xk_sb = x_pool.tile([P, DS * S], bf16, tag="xk")
    xq_sb = x_pool.tile([P, DS * S], bf16, tag="xq")
    xv_sb = x_pool.tile([P, DS * S], bf16, tag="xv")

    qT = [qk_pool.tile([P, S], bf16, tag=f"qT{ft}", name=f"qT{ft}") for ft in range(2)]
    kT = [qk_pool.tile([P, S], bf16, tag=f"kT{ft}", name=f"kT{ft}") for ft in range(2)]

    v_sb = v_pool.tile([P, KS * NH * VW], bf16, tag="v")
    v_view = v_sb[:].rearrange("p (ks h w) -> p ks h w", ks=KS, h=NH)
    nc.gpsimd.memset(v_sb[:], 1.0)

    # ---- PE warmup: contentless matmuls to climb the p-state ramp while the
    # first input DMAs are in flight. Results are discarded.
    nc.gpsimd.memset(wu_sb[:], 0.0)
    for wu in range(20):
        ps = pj_pool.tile([P, QW], f32, tag="pj", name=f"wu{wu}")
        nc.tensor.matmul(ps[:], wu_sb[:, 0:P], wu_sb[:], start=True, stop=True)

    # ---- DMA prologue (SP queue) ----
    def load_w(w_sb, wname, ng):
        nc.sync.dma_start(
            w_sb[:].rearrange("p (g f) -> p g f", g=ng),
            t[wname].ap().rearrange("(g p) f -> p g f", p=P),
        )

    def load_x_chunk(x_sb, xname, c):
        nc.sync.dma_start(
            x_sb[:].rearrange("p (ds q) -> p ds q", ds=DS)[:, :, c * QW:(c + 1) * QW],
            t[xname].ap().rearrange("(ds p) q -> p ds q", p=P)[:, :, c * QW:(c + 1) * QW],
        )

    nm_tiles = {}

    def load_nm(ph, qq):
        # mask slice for quarter qq: [P, KS, QW], two DMAs of 8 key slices each
        nmt = nm_pool.tile([P, KS * QW], bf16, tag="nm", name=f"nm{ph}_{qq}")
        for half in range(2):
            nc.sync.dma_start(
                nmt[:].rearrange("p (ks q) -> p ks q", ks=KS)[:, half * 8:(half + 1) * 8, :],
                t["nmT"].ap().rearrange("(ks p) q -> p ks q", p=P)[
                    :, half * 8:(half + 1) * 8, qq * QW:(qq + 1) * QW],
            )
        nm_tiles[qq] = nmt

    load_w(wk_sb, "wkT", DS)
    load_x_chunk(xk_sb, "xkT", 0)
    load_w(wq_sb, "wqT", DS)
    load_x_chunk(xq_sb, "xqT", 0)
    nc.sync.dma_start(bk_sb[:], t["bk"].ap().rearrange("(ft p) one -> p (ft one)", p=P))
    nc.sync.dma_start(bq_sb[:], t["bq"].ap().rearrange("(ft p) one -> p (ft one)", p=P))
    load_nm(0, 0)
    load_w(wv_sb, "wvT", DS)
    load_x_chunk(xv_sb, "xvT", 0)
    nc.sync.dma_start(ident_sb[:], t["ident"].ap())
    load_x_chunk(xk_sb, "xkT", 1)
    load_x_chunk(xq_sb, "xqT", 1)
    load_x_chunk(xv_sb, "xvT", 1)
    load_nm(0, 1)
    load_x_chunk(xk_sb, "xkT", 2)
    load_x_chunk(xq_sb, "xqT", 2)
    load_x_chunk(xv_sb, "xvT", 2)
    load_x_chunk(xk_sb, "xkT", 3)
    load_x_chunk(xq_sb, "xqT", 3)
    load_x_chunk(xv_sb, "xvT", 3)
    load_w(wom_sb, "womT", 2)

    # ---- projection work units (also used as woven "foreign" chunks) ----
    def qk_chunk(w_sb, x_sb, out_tiles, b_sb, ft, tc4):
        ps = pj_pool.tile([P, QW], f32, tag="pj")
        for ds in range(DS):
            nc.tensor.matmul(
                ps[:],
                w_sb[:, ds * F + ft * P: ds * F + (ft + 1) * P],
                x_sb[:, ds * S + tc4 * QW: ds * S + (tc4 + 1) * QW],
                start=(ds == 0),
                stop=(ds == DS - 1),
            )
        nc.vector.tensor_scalar_add(
            out_tiles[ft][:, tc4 * QW:(tc4 + 1) * QW], ps[:], b_sb[:, ft:ft + 1]
        )

    def vp_chunk(hp, tp):
        # v projection for head pair hp, token slices 2*tp, 2*tp+1
        for tt in (2 * tp, 2 * tp + 1):
            ps = pj_pool.tile([P, QW], f32, tag="pj")
            for ds in range(DS):
                nc.tensor.matmul(
                    ps[:, 0:P],
                    xv_sb[:, ds * S + tt * P: ds * S + (tt + 1) * P],
                    wv_sb[:, ds * F + hp * P: ds * F + (hp + 1) * P],
                    start=(ds == 0),
                    stop=(ds == DS - 1),
                )
            nc.gpsimd.tensor_copy(
                v_view[:, tt, 2 * hp:2 * hp + 2, 0:DH],
                ps[:, 0:P].rearrange("p (h f) -> p h f", h=2),
            )

    # ---- foreign-work weave with deadline forcing ----
    foreign = []
    done = set()

    def _run(key):
        kind = key[0]
        if kind == "k":
            qk_chunk(wk_sb, xk_sb, kT, bk_sb, key[1], key[2])
        elif kind == "q":
            qk_chunk(wq_sb, xq_sb, qT, bq_sb, key[1], key[2])
        elif kind == "v":
            vp_chunk(key[1], key[2])
        done.add(key)

    def pump(n=1):
        while n > 0 and foreign:
            _run(foreign.pop(0))
            n -= 1

    def require(key):
        while key not in done:
            assert foreign, f"foreign queue empty but {key} missing"
            k = foreign.pop(0)
            _run(k)

    # prologue compute: first key chunk + first query quarter of ft0
    qk_chunk(wk_sb, xk_sb, kT, bk_sb, 0, 0)
    qk_chunk(wq_sb, xq_sb, qT, bq_sb, 0, 0)
    done.add(("k", 0, 0))
    done.add(("q", 0, 0))

    foreign.extend(
        [("k", 0, 1), ("v", 0, 0), ("k", 0, 2), ("v", 0, 1), ("k", 0, 3)]
        + [("v", 0, tp) for tp in range(2, 8)]
        + [("q", 0, 1), ("k", 1, 0), ("q", 0, 2), ("k", 1, 1), ("q", 0, 3),
           ("k", 1, 2), ("k", 1, 3), ("q", 1, 0)]
        + [("v", 1, tp) for tp in range(4)]
        + [("q", 1, 1), ("v", 1, 4), ("v", 1, 5), ("q", 1, 2),
           ("v", 1, 6), ("v", 1, 7), ("q", 1, 3)]
    )

    # ---- attention session: one (head, quarter) ----
    ctxT = {}   # (qq, fs) -> tile

    def session(h, qq, quota):
        ft, ro = h // 2, (h % 2) * DH
        hp = h // 2
        require(("q", ft, qq))
        if (qq, ft) not in ctxT:
            ctxT[(qq, ft)] = ct_pool.tile([P, QW], bf16, tag=f"ct{ft}", name=f"ct{ft}_{qq}")
        cx = cx_pool.tile([P, 4 * VW], f32, tag="cx")
        cxv = cx[:].rearrange("p (b w) -> p b w", b=4)
        nmv = nm_tiles[qq][:].rearrange("p (ks q) -> p ks q", ks=KS)
        pts = [None] * NPAIR
        pumped = 0
        for p in range(NPAIR + DEPTH):
            if p < NPAIR:
                require(("k", ft, p // 2))
                ps = sc_pool.tile([P, 2 * QW], f32, tag="sc")
                for j in (0, 1):
                    ks = 2 * p + j
                    nc.tensor.matmul(
                        ps[:, j * QW:(j + 1) * QW],
                        kT[ft][ro:ro + DH, ks * P:(ks + 1) * P],
                        qT[ft][ro:ro + DH, qq * QW:(qq + 1) * QW],
                        start=True,
                        stop=True,
                    )
                pt = pt_pool.tile([P, 2 * QW], bf16, tag="pt")
                nc.scalar.activation(pt[:], ps[:], Exp, scale=0.125)
                ptv = pt[:].rearrange("p (j q) -> p j q", j=2)
                nc.vector.tensor_mul(ptv, ptv, nmv[:, 2 * p:2 * p + 2, :])
                pts[p] = pt
                if dbg is not None and h == 0:
                    nc.sync.dma_start(
                        dbg["pT0"].ap().rearrange(
                            "(pp two pr) q -> pr pp two q", pr=P, two=2)
                        [:, p, :, qq * QW:(qq + 1) * QW],
                        pt[:].rearrange("p (two q) -> p two q", two=2),
                    )
                if pumped < quota:
                    pump(1)
                    pumped += 1
            c = p - DEPTH
            if c >= 0:
                require(("v", hp, c))
                pt = pts[c]
                for j in (0, 1):
                    ks = 2 * c + j
                    for qb in range(4):
                        nc.tensor.matmul(
                            cxv[:, qb, :],
                            pt[:, j * QW + qb * P: j * QW + (qb + 1) * P],
                            v_view[:, ks, h, :],
                            start=(ks == 0),
                            stop=(ks == KS - 1),
                        )
        # normalize by the ones-column denominator, transpose to [f, q]
        tr = tr_pool.tile([DH, 4 * P], bf16, tag="tr")
        for qb in range(4):
            rs = rs_pool.tile([P, 1], f32, tag="rs")
            nc.vector.reciprocal(rs[:], cxv[:, qb, DH:DH + 1])
            cn_t = cn_pool.tile([P, DH], bf16, tag="cn")
            nc.vector.tensor_scalar_mul(cn_t[:], cxv[:, qb, 0:DH], rs[:])
            nc.tensor.matmul(
                tr[:, qb * P:(qb + 1) * P], cn_t[:], ident_sb[:], is_transpose=True
            )
        nc.vector.tensor_copy(ctxT[(qq, ft)][ro:ro + DH, :], tr[:])

    op_pool = None

    def outproj(qq):
        for ot in range(8):
            ps = op_pool.tile([P, QW], f32, tag="op")
            for fs in range(2):
                nc.tensor.matmul(
                    ps[:],
                    wom_sb[:, fs * D + ot * P: fs * D + (ot + 1) * P],
                    ctxT[(qq, fs)][:],
                    start=(fs == 0),
                    stop=(fs == 1),
                )
            ob = ob_pool.tile([P, QW], bf16, tag="ob")
            nc.gpsimd.tensor_copy(ob[:], ps[:])
            nc.sync.dma_start(
                t["outT"].ap()[ot * P:(ot + 1) * P, qq * QW:(qq + 1) * QW], ob[:]
            )

    # ---- phase 1: heads 0,1 over quarters (nm loaded once per quarter) ----
    for qq in range(NQ):
        if 1 <= qq < NQ - 1:
            load_nm(0, qq + 1)   # prefetch next quarter's mask
        if qq == NQ - 1:
            load_nm(1, 0)        # prefetch phase 2's first mask reload
        session(0, qq, quota=8 if qq == 0 else 2)
        session(1, qq, quota=2)

    # ---- phase 2: heads 2,3 + output projection per quarter ----
    op_cm = None
    for qq in range(NQ):
        if qq < NQ - 1:
            load_nm(1, qq + 1)   # prefetch next quarter's mask reload
        session(2, qq, quota=4)
        session(3, qq, quota=4)
        if qq == 0:
            # all projection work is done; hand pj's PSUM banks to outproj
            while foreign:
                pump(1)
            pj_cm.__exit__(None, None, None)
            op_cm = tc.tile_pool(name="op", bufs=2, space="PSUM")
            op_pool = op_cm.__enter__()
        outproj(qq)

    if dbg is not None:
        for ft in range(2):
            nc.sync.dma_start(dbg["qT"].ap()[ft * P:(ft + 1) * P, :], qT[ft][:])
            nc.sync.dma_start(dbg["kT"].ap()[ft * P:(ft + 1) * P, :], kT[ft][:])
        nc.sync.dma_start(dbg["v"].ap(), v_sb[:])
        for qq in range(NQ):
            for fs in range(2):
                nc.sync.dma_start(
                    dbg["ctxT"].ap()[fs * P:(fs + 1) * P, qq * QW:(qq + 1) * QW],
                    ctxT[(qq, fs)][:],
                )

    assert not foreign, f"undrained foreign work: {foreign}"
    op_cm.__exit__(None, None, None)
    es.close()


@functools.lru_cache(maxsize=1)
def _build(debug=False):
    import concourse.bacc as bacc
    import concourse.mybir as mybir
    import concourse.tile as tile

    bf16 = mybir.dt.bfloat16
    f32 = mybir.dt.float32

    nc = bacc.Bacc("TRN2", target_bir_lowering=False, debug=False, num_devices=NCORES)
    t = {
        "xqT": nc.dram_tensor("xqT", (D, S), bf16, kind="ExternalInput"),
        "xkT": nc.dram_tensor("xkT", (D, S), bf16, kind="ExternalInput"),
        "xvT": nc.dram_tensor("xvT", (D, S), bf16, kind="ExternalInput"),
        "wqT": nc.dram_tensor("wqT", (D, F), bf16, kind="ExternalInput"),
        "wkT": nc.dram_tensor("wkT", (D, F), bf16, kind="ExternalInput"),
        "wvT": nc.dram_tensor("wvT", (D, F), bf16, kind="ExternalInput"),
        "womT": nc.dram_tensor("womT", (F, D), bf16, kind="ExternalInput"),
        "nmT": nc.dram_tensor("nmT", (S, S), bf16, kind="ExternalInput"),
        "bq": nc.dram_tensor("bq", (F, 1), f32, kind="ExternalInput"),
        "bk": nc.dram_tensor("bk", (F, 1), f32, kind="ExternalInput"),
        "ident": nc.dram_tensor("ident", (P, P), bf16, kind="ExternalInput"),
        "outT": nc.dram_tensor("outT", (D, S), bf16, kind="ExternalOutput"),
    }
    dbg = None
    if debug:
        dbg = {
            "qT": nc.dram_tensor("dbg_qT", (F, S), bf16, kind="ExternalOutput"),
            "kT": nc.dram_tensor("dbg_kT", (F, S), bf16, kind="ExternalOutput"),
            "v": nc.dram_tensor("dbg_v", (P, KS * NH * VW), bf16, kind="ExternalOutput"),
            "pT0": nc.dram_tensor("dbg_pT0", (S, S), bf16, kind="ExternalOutput"),
            "ctxT": nc.dram_tensor("dbg_ctxT", (F, S), bf16, kind="ExternalOutput"),
        }
    with tile.TileContext(nc) as tc:
        _emit(nc, tc, t, dbg)
    nc.compile()
    return nc


def _prep_core_inputs(c, Q, K, V, mask, Wq, bq, Wk, bk, Wv, Wo, _cache={}):
    import ml_dtypes

    bf = ml_dtypes.bfloat16
    b, g = divmod(c, GROUPS)
    bkey = ("batch", b, id(Q))
    if bkey not in _cache:
        _cache.clear()
        for bb in range(B):
            nm = 1.0 - mask[bb, 0].astype(np.float32)
            _cache[("batch", bb, id(Q))] = {
                "xqT": Q[bb].T.astype(bf),
                "xkT": K[bb].T.astype(bf),
                "xvT": V[bb].T.astype(bf),
                "nmT": nm.T.astype(bf),
            }
    fsl = slice(g * F, (g + 1) * F)
    return {
        **_cache[bkey],
        "wqT": Wq[fsl, :].T.astype(bf),
        "wkT": Wk[fsl, :].T.astype(bf),
        "wvT": Wv[fsl, :].T.astype(bf),
        "womT": Wo[:, fsl].T.astype(bf),
        "bq": bq[fsl].reshape(F, 1).astype(np.float32),
        "bk": bk[fsl].reshape(F, 1).astype(np.float32),
        "ident": np.eye(P, dtype=bf),
    }


def kernel(Q, K, V, mask, Wq, bq, Wk, bk, Wv, bv, Wo, bo, _trace=False, _tmpdir=None):
    from concourse.bass_utils import run_bass_kernel_spmd

    Q, K, V = np.asarray(Q, np.float32), np.asarray(K, np.float32), np.asarray(V, np.float32)
    mask = np.asarray(mask)
    Wq, Wk, Wv, Wo = (np.asarray(w, np.float32) for w in (Wq, Wk, Wv, Wo))
    bq, bk, bv, bo = (np.asarray(x, np.float32) for x in (bq, bk, bv, bo))

    nc = _build()
    in_maps = [_prep_core_inputs(c, Q, K, V, mask, Wq, bq, Wk, bk, Wv, Wo) for c in range(NCORES)]
    kw = {}
    if _trace:
        kw = dict(trace=True, tmpdir=_tmpdir)
    res = run_bass_kernel_spmd(nc, in_maps, core_ids=list(range(NCORES)), **kw)

    const = (Wo @ bv + bo).astype(np.float32)  # softmax rows sum to 1 -> bv enters linearly
    out = np.empty((B, S, D), np.float32)
    for b in range(B):
        acc = res.results[b * GROUPS]["outT"].astype(np.float32)
        for g in range(1, GROUPS):
            acc = acc + res.results[b * GROUPS + g]["outT"].astype(np.float32)
        out[b] = acc.T + const
    if _trace:
        kernel._last_results = res
    return out


# revision 17
# speedup vs baseline: 1.0745x; 1.0131x over previous
"""Multi-head attention (B=2, S=2048, D=1024, H=16) on 8 trn2 NeuronCores.

Sharding: data-parallel over batch (2) x tensor-parallel over head-groups (4).
Core c handles batch c//4, heads [4*(c%4), 4*(c%4)+4).  Each core computes
q/k/v projections for its 256 head-features, masked softmax attention for its
4 heads, and the row-shard of the output projection; partial outputs are
summed on the host during the gather step.

Schedule: attention is blocked into 4 query-quarters of 512.  The ctx matmul
runs with pt as the stationary operand (q on the output partitions, head
features on the free dim), which halves its PE cost versus the v-stationary
orientation; the softmax denominator rides along as a 65th v column and is
divided out with a per-partition reciprocal before a PE transpose puts ctx
back into [feature, q] layout for the output projection.  Projections for
later head groups are woven between attention score pairs so the activation
engine (exp) starts early and stays fed.
"""

import sys
import functools
from contextlib import ExitStack

sys.path.insert(0, "/opt/trn_rl_repo")

import numpy as np

B, S, D, H = 2, 2048, 1024, 16
DH = 64
P = 128
NCORES = 8
GROUPS = 4            # head groups = cores per batch
NH = H // GROUPS      # heads per core = 4
F = NH * DH           # head features per core = 256
KS = S // P           # 16 key slices
DS = D // P           # 8 d_model slices
QW = 512              # query quarter width
NQ = S // QW          # 4 quarters
NPAIR = KS // 2       # 8 key-slice pairs
VW = DH + 1           # v width per head incl. ones column = 65
DEPTH = 4             # ctx lag behind scores, in key-slice pairs


def _emit(nc, tc, t, dbg=None):
    import concourse.mybir as mybir
    bf16 = mybir.dt.bfloat16
    f32 = mybir.dt.float32
    Exp = mybir.ActivationFunctionType.Exp

    es = ExitStack()
    ep = lambda cm: es.enter_context(cm)
    const_pool = ep(tc.tile_pool(name="const", bufs=1))
    w_pool = ep(tc.tile_pool(name="w", bufs=1))
    x_pool = ep(tc.tile_pool(name="x", bufs=1))
    qk_pool = ep(tc.tile_pool(name="qk", bufs=1))
    v_pool = ep(tc.tile_pool(name="v", bufs=1))
    nm_pool = ep(tc.tile_pool(name="nm", bufs=2))
    pt_pool = ep(tc.tile_pool(name="pt", bufs=10))
    cn_pool = ep(tc.tile_pool(name="cn", bufs=48))
    rs_pool = ep(tc.tile_pool(name="rs", bufs=4))
    ct_pool = ep(tc.tile_pool(name="ct", bufs=2))
    ob_pool = ep(tc.tile_pool(name="ob", bufs=4))
    # PSUM banks: sc 2x2 + cx 2x1 + pj 2x1 = 8 during the projection phase;
    # pj's two banks are handed to tr (1) + op (1) once projections finish.
    # pj is opened last so its mid-emit release keeps the pool stack LIFO.
    sc_pool = ep(tc.tile_pool(name="sc", bufs=2, space="PSUM"))
    cx_pool = ep(tc.tile_pool(name="cx", bufs=2, space="PSUM"))
    tr_pool = None
    pj_cm = tc.tile_pool(name="pj", bufs=2, space="PSUM")
    pj_pool = pj_cm.__enter__()

    # ---- SBUF tiles ----
    bq_sb = const_pool.tile([P, 2], f32, tag="bq")
    bk_sb = const_pool.tile([P, 2], f32, tag="bk")
    ident_sb = const_pool.tile([P, P], bf16, tag="ident")
    wu_sb = const_pool.tile([P, QW], bf16, tag="wu")  # warmup garbage

    wq_sb = w_pool.tile([P, DS * F], bf16, tag="wq")
    wk_sb = w_pool.tile([P, DS * F], bf16, tag="wk")
    wv_sb = w_pool.tile([P, DS * F], bf16, tag="wv")
    wom_sb = w_pool.tile([P, 2 * D], bf16, tag="wom")

    xk_sb = x_pool.tile([P, DS * S], bf16, tag="xk")
    xq_sb = x_pool.tile([P, DS * S], bf16, tag="xq")
    xv_sb = x_pool.tile([P, DS * S], bf16, tag="xv")

    qT = [qk_pool.tile([P, S], bf16, tag=f"qT{ft}", name=f"qT{ft}") for ft in range(2)]
    kT = [qk_pool.tile([P, S], bf16, tag=f"kT{ft}", name=f"kT{ft}") for ft in range(2)]

    v_sb = v_pool.tile([P, KS * NH * VW], bf16, tag="v")
    v_view = v_sb[:].rearrange("p (ks h w) -> p ks h w", ks=KS, h=NH)
    nc.gpsimd.memset(v_sb[:], 1.0)

    # ---- PE warmup: contentless matmuls to climb the p-state ramp while the
    # first input DMAs are in flight. Results are discarded.
    nc.gpsimd.memset(wu_sb[:], 0.0)
    for wu in range(20):
        ps = pj_pool.tile([P, QW], f32, tag="pj", name=f"wu{wu}")
        nc.tensor.matmul(ps[:], wu_sb[:, 0:P], wu_sb[:], start=True, stop=True)

    # ---- DMA prologue (SP queue) ----
    def load_w(w_sb, wname, ng):
        nc.sync.dma_start(
            w_sb[:].rearrange("p (g f) -> p g f", g=ng),
            t[wname].ap().rearrange("(g p) f -> p g f", p=P),
        )

    def load_x_chunk(x_sb, xname, c):
        nc.sync.dma_start(
            x_sb[:].rearrange("p (ds q) -> p ds q", ds=DS)[:, :, c * QW:(c + 1) * QW],
            t[xname].ap().rearrange("(ds p) q -> p ds q", p=P)[:, :, c * QW:(c + 1) * QW],
        )

    nm_tiles = {}

    def load_nm(ph, qq):
        # mask slice for quarter qq: [P, KS, QW], two DMAs of 8 key slices each
        nmt = nm_pool.tile([P, KS * QW], bf16, tag="nm", name=f"nm{ph}_{qq}")
        for half in range(2):
            nc.sync.dma_start(
                nmt[:].rearrange("p (ks q) -> p ks q", ks=KS)[:, half * 8:(half + 1) * 8, :],
                t["nmT"].ap().rearrange("(ks p) q -> p ks q", p=P)[
                    :, half * 8:(half + 1) * 8, qq * QW:(qq + 1) * QW],
            )
        nm_tiles[qq] = nmt

    # Ordered by first consumption on the serial DMA device: the first
    # session needs kft0/qft0 inputs, then the q0 mask, then ALL of xv (its
    # v-projection weave covers every token slice).
    load_w(wk_sb, "wkT", DS)
    load_x_chunk(xk_sb, "xkT", 0)
    load_w(wq_sb, "wqT", DS)
    load_x_chunk(xq_sb, "xqT", 0)
    nc.sync.dma_start(bk_sb[:], t["bk"].ap().rearrange("(ft p) one -> p (ft one)", p=P))
    nc.sync.dma_start(bq_sb[:], t["bq"].ap().rearrange("(ft p) one -> p (ft one)", p=P))
    load_nm(0, 0)
    load_w(wv_sb, "wvT", DS)
    load_x_chunk(xv_sb, "xvT", 0)
    load_x_chunk(xv_sb, "xvT", 1)
    load_x_chunk(xv_sb, "xvT", 2)
    load_x_chunk(xv_sb, "xvT", 3)
    nc.sync.dma_start(ident_sb[:], t["ident"].ap())
    load_nm(0, 1)
    load_x_chunk(xq_sb, "xqT", 1)
    load_x_chunk(xk_sb, "xkT", 1)
    load_x_chunk(xk_sb, "xkT", 2)
    load_x_chunk(xk_sb, "xkT", 3)
    load_x_chunk(xq_sb, "xqT", 2)
    load_x_chunk(xq_sb, "xqT", 3)
    load_w(wom_sb, "womT", 2)

    # ---- projection work units (also used as woven "foreign" chunks) ----
    def qk_chunk(w_sb, x_sb, out_tiles, b_sb, ft, tc4):
        ps = pj_pool.tile([P, QW], f32, tag="pj")
        for ds in range(DS):
            nc.tensor.matmul(
                ps[:],
                w_sb[:, ds * F + ft * P: ds * F + (ft + 1) * P],
                x_sb[:, ds * S + tc4 * QW: ds * S + (tc4 + 1) * QW],
                start=(ds == 0),
                stop=(ds == DS - 1),
            )
        nc.vector.tensor_scalar_add(
            out_tiles[ft][:, tc4 * QW:(tc4 + 1) * QW], ps[:], b_sb[:, ft:ft + 1]
        )

    def vp_chunk(hp, tp):
        # v projection for head pair hp, token slices 2*tp, 2*tp+1
        for tt in (2 * tp, 2 * tp + 1):
            ps = pj_pool.tile([P, QW], f32, tag="pj")
            for ds in range(DS):
                nc.tensor.matmul(
                    ps[:, 0:P],
                    xv_sb[:, ds * S + tt * P: ds * S + (tt + 1) * P],
                    wv_sb[:, ds * F + hp * P: ds * F + (hp + 1) * P],
                    start=(ds == 0),
                    stop=(ds == DS - 1),
                )
            nc.gpsimd.tensor_copy(
                v_view[:, tt, 2 * hp:2 * hp + 2, 0:DH],
                ps[:, 0:P].rearrange("p (h f) -> p h f", h=2),
            )

    # ---- foreign-work weave with deadline forcing ----
    foreign = []
    done = set()

    def _run(key):
        kind = key[0]
        if kind == "k":
            qk_chunk(wk_sb, xk_sb, kT, bk_sb, key[1], key[2])
        elif kind == "q":
            qk_chunk(wq_sb, xq_sb, qT, bq_sb, key[1], key[2])
        elif kind == "v":
            vp_chunk(key[1], key[2])
        done.add(key)

    def pump(n=1):
        while n > 0 and foreign:
            _run(foreign.pop(0))
            n -= 1

    def require(key):
        while key not in done:
            assert foreign, f"foreign queue empty but {key} missing"
            _run(foreign.pop(0))

    # prologue compute: first key chunk + first query quarter of ft0
    qk_chunk(wk_sb, xk_sb, kT, bk_sb, 0, 0)
    qk_chunk(wq_sb, xq_sb, qT, bq_sb, 0, 0)
    done.add(("k", 0, 0))
    done.add(("q", 0, 0))

    foreign.extend(
        [("k", 0, 1), ("k", 0, 2), ("k", 0, 3), ("q", 0, 1)]
        + [("v", 0, tp) for tp in range(8)]
        + [("q", 0, 2), ("q", 0, 3),
           ("k", 1, 0), ("k", 1, 1), ("k", 1, 2), ("k", 1, 3), ("q", 1, 0)]
        + [("v", 1, tp) for tp in range(8)]
        + [("q", 1, 1), ("q", 1, 2), ("q", 1, 3)]
    )

    # ---- output-projection work units (woven after each quarter's last norm)
    ctxT = {}       # (qq, fs) -> tile
    cn_tiles = {}   # (h, qq) -> [4 normalized ctx blocks in [q, f] layout]
    opq = []        # pending transpose / output-projection units
    op_pool = None

    def transp_unit(h, qq):
        ft, ro = h // 2, (h % 2) * DH
        if (qq, ft) not in ctxT:
            ctxT[(qq, ft)] = ct_pool.tile([P, QW], bf16, tag=f"ct{ft}", name=f"ct{ft}_{qq}")
        trt = tr_pool.tile([DH, 4 * P], bf16, tag="tr")
        for qb in range(4):
            nc.tensor.matmul(
                trt[:, qb * P:(qb + 1) * P], cn_tiles[(h, qq)][qb][:], ident_sb[:],
                is_transpose=True,
            )
        nc.vector.tensor_copy(ctxT[(qq, ft)][ro:ro + DH, :], trt[:])

    def op_unit(qq, ot):
        ps = op_pool.tile([P, QW], f32, tag="op")
        for fs in range(2):
            nc.tensor.matmul(
                ps[:],
                wom_sb[:, fs * D + ot * P: fs * D + (ot + 1) * P],
                ctxT[(qq, fs)][:],
                start=(fs == 0),
                stop=(fs == 1),
            )
        ob = ob_pool.tile([P, QW], bf16, tag="ob")
        nc.gpsimd.tensor_copy(ob[:], ps[:])
        nc.sync.dma_start(
            t["outT"].ap()[ot * P:(ot + 1) * P, qq * QW:(qq + 1) * QW], ob[:]
        )

    def run_opq(n):
        while n > 0 and opq:
            kind, h_or_ot, qq = opq.pop(0)
            if kind == "tr":
                transp_unit(h_or_ot, qq)
            else:
                op_unit(qq, h_or_ot)
            n -= 1

    # ---- session machinery: scores/exp/mask for (h, qq); the PREVIOUS
    # session's ctx matmuls and normalization weave into this session's slots
    # so the exp stream never waits on ctx inputs.
    def ctx_pair(st, c):
        h = st["h"]
        require(("v", h // 2, c))
        pt = st["pts"][c]
        for j in (0, 1):
            ks = 2 * c + j
            for qb in range(4):
                nc.tensor.matmul(
                    st["cxv"][:, qb, :],
                    pt[:, j * QW + qb * P: j * QW + (qb + 1) * P],
                    v_view[:, ks, h, :],
                    start=(ks == 0),
                    stop=(ks == KS - 1),
                )
        st["pts"][c] = None

    def norm(st):
        h, qq, cxv = st["h"], st["qq"], st["cxv"]
        blocks = []
        for qb in range(4):
            rs = rs_pool.tile([P, 1], f32, tag="rs")
            nc.vector.reciprocal(rs[:], cxv[:, qb, DH:DH + 1])
            cn_t = cn_pool.tile([P, DH], bf16, tag="cn", name=f"cn{h}_{qq}_{qb}")
            nc.vector.tensor_scalar_mul(cn_t[:], cxv[:, qb, 0:DH], rs[:])
            blocks.append(cn_t)
        cn_tiles[(h, qq)] = blocks
        if st["h"] == 3:
            # quarter complete: queue its transposes + output projection
            for hh in range(4):
                opq.append(("tr", hh, qq))
            for ot in range(8):
                opq.append(("op", ot, qq))

    def run_session(h, qq, prev, quota):
        ft, ro = h // 2, (h % 2) * DH
        require(("q", ft, qq))
        cx = cx_pool.tile([P, 4 * VW], f32, tag="cx")
        st = {
            "h": h, "qq": qq, "pts": [None] * NPAIR,
            "cxv": cx[:].rearrange("p (b w) -> p b w", b=4),
        }
        nmv = nm_tiles[qq][:].rearrange("p (ks q) -> p ks q", ks=KS)
        pumped = 0
        for p in range(NPAIR):
            require(("k", ft, p // 2))
            ps = sc_pool.tile([P, 2 * QW], f32, tag="sc")
            for j in (0, 1):
                ks = 2 * p + j
                nc.tensor.matmul(
                    ps[:, j * QW:(j + 1) * QW],
                    kT[ft][ro:ro + DH, ks * P:(ks + 1) * P],
                    qT[ft][ro:ro + DH, qq * QW:(qq + 1) * QW],
                    start=True,
                    stop=True,
                )
            pt = pt_pool.tile([P, 2 * QW], bf16, tag="pt")
            nc.scalar.activation(pt[:], ps[:], Exp, scale=0.125)
            ptv = pt[:].rearrange("p (j q) -> p j q", j=2)
            nc.vector.tensor_mul(ptv, ptv, nmv[:, 2 * p:2 * p + 2, :])
            st["pts"][p] = pt
            if dbg is not None and h == 0:
                nc.sync.dma_start(
                    dbg["pT0"].ap().rearrange(
                        "(pp two pr) q -> pr pp two q", pr=P, two=2)
                    [:, p, :, qq * QW:(qq + 1) * QW],
                    pt[:].rearrange("p (two q) -> p two q", two=2),
                )
            if prev is not None:
                ctx_pair(prev, p)
                if p == NPAIR - 1:
                    norm(prev)
            if opq:
                run_opq(2)
            elif pumped < quota:
                pump(1)
                pumped += 1
        return st

    def drain_session(st):
        for c in range(NPAIR):
            ctx_pair(st, c)
        norm(st)

    # ---- main schedule: 16 sessions, software-pipelined ----
    order = [(h0 + dh, qq) for ph, h0 in ((1, 0), (2, 2)) for qq in range(NQ)
             for dh in (0, 1)]
    prev = None
    op_cm = tr_cm = None
    for si, (h, qq) in enumerate(order):
        # mask prefetch: one quarter of lead within each phase
        if (h, qq) == (1, 0):
            load_nm(0, 2)
        elif (h, qq) == (1, 1):
            load_nm(0, 3)
        elif (h, qq) == (1, 2):
            load_nm(1, 0)
        elif (h, qq) == (3, 0):
            load_nm(1, 1)
        elif (h, qq) == (3, 1):
            load_nm(1, 2)
        elif (h, qq) == (3, 2):
            load_nm(1, 3)
        if si == 10:
            # norm(3,0) has been emitted (end of session 9): all projection
            # work must be done now; swap pj's PSUM banks to transpose+outproj
            while foreign:
                pump(1)
            pj_cm.__exit__(None, None, None)
            tr_cm = tc.tile_pool(name="tr", bufs=1, space="PSUM")
            tr_pool = tr_cm.__enter__()
            op_cm = tc.tile_pool(name="op", bufs=1, space="PSUM")
            op_pool = op_cm.__enter__()
        prev = run_session(h, qq, prev, quota=5 if si == 0 else 2)
    drain_session(prev)

    # tail: last quarter's transposes, then a double-buffered outproj burst
    while opq and opq[0][0] == "tr":
        kind, hh, qq = opq.pop(0)
        transp_unit(hh, qq)
    op_cm.__exit__(None, None, None)
    tr_cm.__exit__(None, None, None)
    op_cm = tc.tile_pool(name="op2", bufs=2, space="PSUM")
    op_pool = op_cm.__enter__()
    run_opq(len(opq))

    if dbg is not None:
        for ft in range(2):
            nc.sync.dma_start(dbg["qT"].ap()[ft * P:(ft + 1) * P, :], qT[ft][:])
            nc.sync.dma_start(dbg["kT"].ap()[ft * P:(ft + 1) * P, :], kT[ft][:])
        nc.sync.dma_start(dbg["v"].ap(), v_sb[:])
        for qq in range(NQ):
            for fs in range(2):
                nc.sync.dma_start(
                    dbg["ctxT"].ap()[fs * P:(fs + 1) * P, qq * QW:(qq + 1) * QW],
                    ctxT[(qq, fs)][:],
                )

    assert not foreign and not opq, f"undrained work: {foreign} {opq}"
    op_cm.__exit__(None, None, None)
    es.close()


@functools.lru_cache(maxsize=1)
def _build(debug=False):
    import concourse.bacc as bacc
    import concourse.mybir as mybir
    import concourse.tile as tile

    bf16 = mybir.dt.bfloat16
    f32 = mybir.dt.float32

    nc = bacc.Bacc("TRN2", target_bir_lowering=False, debug=False, num_devices=NCORES)
    t = {
        "xqT": nc.dram_tensor("xqT", (D, S), bf16, kind="ExternalInput"),
        "xkT": nc.dram_tensor("xkT", (D, S), bf16, kind="ExternalInput"),
        "xvT": nc.dram_tensor("xvT", (D, S), bf16, kind="ExternalInput"),
        "wqT": nc.dram_tensor("wqT", (D, F), bf16, kind="ExternalInput"),
        "wkT": nc.dram_tensor("wkT", (D, F), bf16, kind="ExternalInput"),
        "wvT": nc.dram_tensor("wvT", (D, F), bf16, kind="ExternalInput"),
        "womT": nc.dram_tensor("womT", (F, D), bf16, kind="ExternalInput"),
        "nmT": nc.dram_tensor("nmT", (S, S), bf16, kind="ExternalInput"),
        "bq": nc.dram_tensor("bq", (F, 1), f32, kind="ExternalInput"),
        "bk": nc.dram_tensor("bk", (F, 1), f32, kind="ExternalInput"),
        "ident": nc.dram_tensor("ident", (P, P), bf16, kind="ExternalInput"),
        "outT": nc.dram_tensor("outT", (D, S), bf16, kind="ExternalOutput"),
    }
    dbg = None
    if debug:
        dbg = {
            "qT": nc.dram_tensor("dbg_qT", (F, S), bf16, kind="ExternalOutput"),
            "kT": nc.dram_tensor("dbg_kT", (F, S), bf16, kind="ExternalOutput"),
            "v": nc.dram_tensor("dbg_v", (P, KS * NH * VW), bf16, kind="ExternalOutput"),
            "pT0": nc.dram_tensor("dbg_pT0", (S, S), bf16, kind="ExternalOutput"),
            "ctxT": nc.dram_tensor("dbg_ctxT", (F, S), bf16, kind="ExternalOutput"),
        }
    with tile.TileContext(nc) as tc:
        _emit(nc, tc, t, dbg)
    nc.compile()
    return nc


def _prep_core_inputs(c, Q, K, V, mask, Wq, bq, Wk, bk, Wv, Wo, _cache={}):
    import ml_dtypes

    bf = ml_dtypes.bfloat16
    b, g = divmod(c, GROUPS)
    bkey = ("batch", b, id(Q))
    if bkey not in _cache:
        _cache.clear()
        for bb in range(B):
            nm = 1.0 - mask[bb, 0].astype(np.float32)
            _cache[("batch", bb, id(Q))] = {
                "xqT": Q[bb].T.astype(bf),
                "xkT": K[bb].T.astype(bf),
                "xvT": V[bb].T.astype(bf),
                "nmT": nm.T.astype(bf),
            }
    fsl = slice(g * F, (g + 1) * F)
    return {
        **_cache[bkey],
        "wqT": Wq[fsl, :].T.astype(bf),
        "wkT": Wk[fsl, :].T.astype(bf),
        "wvT": Wv[fsl, :].T.astype(bf),
        "womT": Wo[:, fsl].T.astype(bf),
        "bq": bq[fsl].reshape(F, 1).astype(np.float32),
        "bk": bk[fsl].reshape(F, 1).astype(np.float32),
        "ident": np.eye(P, dtype=bf),
    }


def kernel(Q, K, V, mask, Wq, bq, Wk, bk, Wv, bv, Wo, bo, _trace=False, _tmpdir=None):
    from concourse.bass_utils import run_bass_kernel_spmd

    Q, K, V = np.asarray(Q, np.float32), np.asarray(K, np.float32), np.asarray(V, np.float32)
    mask = np.asarray(mask)
    Wq, Wk, Wv, Wo = (np.asarray(w, np.float32) for w in (Wq, Wk, Wv, Wo))
    bq, bk, bv, bo = (np.asarray(x, np.float32) for x in (bq, bk, bv, bo))

    nc = _build()
    in_maps = [_prep_core_inputs(c, Q, K, V, mask, Wq, bq, Wk, bk, Wv, Wo) for c in range(NCORES)]
    kw = {}
    if _trace:
        kw = dict(trace=True, tmpdir=_tmpdir)
    res = run_bass_kernel_spmd(nc, in_maps, core_ids=list(range(NCORES)), **kw)

    const = (Wo @ bv + bo).astype(np.float32)  # softmax rows sum to 1 -> bv enters linearly
    out = np.empty((B, S, D), np.float32)
    for b in range(B):
        acc = res.results[b * GROUPS]["outT"].astype(np.float32)
        for g in range(1, GROUPS):
            acc = acc + res.results[b * GROUPS + g]["outT"].astype(np.float32)
        out[b] = acc.T + const
    if _trace:
        kernel._last_results = res
    return out


# revision 73
# speedup vs baseline: 1.1674x; 1.0864x over previous
"""Multi-head attention (B=2, S=2048, D=1024, H=16) on 8 trn2 NeuronCores.

Sharding: data-parallel over batch (2) x tensor-parallel over head-groups (4).
Core c handles batch c//4, heads [4*(c%4), 4*(c%4)+4).  Each core computes
q/k/v projections for its 256 head-features, masked softmax attention for its
4 heads, and the row-shard of the output projection; partial outputs are
summed on the host during the gather step.

Schedule: attention is blocked into 4 query-quarters of 512.  The ctx matmul
runs with pt as the stationary operand (q on the output partitions, head
features on the free dim), which halves its PE cost versus the v-stationary
orientation; the softmax denominator rides along as a 65th v column and is
divided out with a per-partition reciprocal before a PE transpose puts ctx
back into [feature, q] layout for the output projection.  Projections for
later head groups are woven between attention score pairs so the activation
engine (exp) starts early and stays fed.
"""

import sys
import functools
from contextlib import ExitStack

sys.path.insert(0, "/opt/trn_rl_repo")

import numpy as np

B, S, D, H = 2, 2048, 1024, 16
DH = 64
P = 128
NCORES = 8
GROUPS = 4            # head groups = cores per batch
NH = H // GROUPS      # heads per core = 4
F = NH * DH           # head features per core = 256
KS = S // P           # 16 key slices
DS = D // P           # 8 d_model slices
QW = 512              # query quarter width
NQ = S // QW          # 4 quarters
NPAIR = KS // 2       # 8 key-slice pairs
VW = DH + 1           # v width per head incl. ones column = 65
DEPTH = 4             # ctx lag behind scores, in key-slice pairs


def _emit(nc, tc, t, dbg=None):
    import concourse.mybir as mybir
    bf16 = mybir.dt.bfloat16
    f32 = mybir.dt.float32
    Exp = mybir.ActivationFunctionType.Exp

    es = ExitStack()
    ep = lambda cm: es.enter_context(cm)
    const_pool = ep(tc.tile_pool(name="const", bufs=1))
    w_pool = ep(tc.tile_pool(name="w", bufs=1))
    x_pool = ep(tc.tile_pool(name="x", bufs=1))
    qk_pool = ep(tc.tile_pool(name="qk", bufs=1))
    v_pool = ep(tc.tile_pool(name="v", bufs=1))
    nm_pool = ep(tc.tile_pool(name="nm", bufs=2))
    pt_pool = ep(tc.tile_pool(name="pt", bufs=13))
    cn_pool = ep(tc.tile_pool(name="cn", bufs=44))
    rs_pool = ep(tc.tile_pool(name="rs", bufs=2))
    ct_pool = ep(tc.tile_pool(name="ct", bufs=2))
    ob_pool = ep(tc.tile_pool(name="ob", bufs=3))
    # PSUM banks: sc 2x2 + cx 2x1 + pj 2x1 = 8 during the projection phase;
    # pj's two banks are handed to tr (1) + op (1) once projections finish.
    # pj is opened last so its mid-emit release keeps the pool stack LIFO.
    sc_pool = ep(tc.tile_pool(name="sc", bufs=2, space="PSUM"))
    cx_cm = tc.tile_pool(name="cx", bufs=2, space="PSUM")
    cx_pool = cx_cm.__enter__()
    tr_pool = None
    pj_cm = tc.tile_pool(name="pj", bufs=2, space="PSUM")
    pj_pool = pj_cm.__enter__()

    # ---- SBUF tiles ----
    bq_sb = const_pool.tile([P, 2], f32, tag="bq")
    bk_sb = const_pool.tile([P, 2], f32, tag="bk")
    ident_sb = const_pool.tile([P, P], bf16, tag="ident")
    wu_sb = const_pool.tile([P, 256], bf16, tag="wu")  # warmup garbage

    wq_sb = w_pool.tile([P, DS * F], bf16, tag="wq")
    wk_sb = w_pool.tile([P, DS * F], bf16, tag="wk")
    wv_sb = w_pool.tile([P, DS * F], bf16, tag="wv")
    wom_sb = w_pool.tile([P, 2 * D], bf16, tag="wom")

    xk_sb = x_pool.tile([P, DS * S], bf16, tag="xk")
    xq_sb = x_pool.tile([P, DS * S], bf16, tag="xq")
    xv_sb = x_pool.tile([P, DS * S], bf16, tag="xv")

    qT = [qk_pool.tile([P, S], bf16, tag=f"qT{ft}", name=f"qT{ft}") for ft in range(2)]
    kT = [qk_pool.tile([P, S], bf16, tag=f"kT{ft}", name=f"kT{ft}") for ft in range(2)]

    v_sb = v_pool.tile([P, KS * NH * VW], bf16, tag="v")
    v_view = v_sb[:].rearrange("p (ks h w) -> p ks h w", ks=KS, h=NH)
    nc.gpsimd.memset(v_sb[:], 1.0)

    # ---- PE warmup: contentless matmuls to climb the p-state ramp while the
    # first input DMAs are in flight. Results are discarded.
    nc.gpsimd.memset(wu_sb[:], 0.0)
    for wu in range(20):
        ps = pj_pool.tile([P, QW], f32, tag="pj", name=f"wu{wu}")
        nc.tensor.matmul(ps[:, 0:256], wu_sb[:, 0:P], wu_sb[:], start=True, stop=True)

    # ---- DMA prologue (SP queue) ----
    def load_w(w_sb, wname, ng):
        nc.sync.dma_start(
            w_sb[:].rearrange("p (g f) -> p g f", g=ng),
            t[wname].ap().rearrange("(g p) f -> p g f", p=P),
        )

    def load_x_chunk(x_sb, xname, c):
        nc.sync.dma_start(
            x_sb[:].rearrange("p (ds q) -> p ds q", ds=DS)[:, :, c * QW:(c + 1) * QW],
            t[xname].ap().rearrange("(ds p) q -> p ds q", p=P)[:, :, c * QW:(c + 1) * QW],
        )

    nm_tiles = {}

    def load_nm(ph, qq):
        # mask slice for quarter qq: [P, KS, QW], four DMAs of 4 key slices
        # each so early score pairs unblock as soon as their slice lands
        nmt = nm_pool.tile([P, KS * QW], bf16, tag="nm", name=f"nm{ph}_{qq}")
        for quad in range(4):
            nc.sync.dma_start(
                nmt[:].rearrange("p (ks q) -> p ks q", ks=KS)[:, quad * 4:(quad + 1) * 4, :],
                t["nmT"].ap().rearrange("(ks p) q -> p ks q", p=P)[
                    :, quad * 4:(quad + 1) * 4, qq * QW:(qq + 1) * QW],
            )
        nm_tiles[qq] = nmt

    # Ordered by first consumption on the serial DMA device.  The exp stream
    # only needs wk/xk + wq/xq-quarter0; the mask multiply (DVE) and the ctx
    # weave (one session behind) tolerate later arrival, so nm and xv follow
    # the full xk.
    load_w(wk_sb, "wkT", DS)
    load_x_chunk(xk_sb, "xkT", 0)
    load_w(wq_sb, "wqT", DS)
    load_x_chunk(xq_sb, "xqT", 0)
    nc.sync.dma_start(bk_sb[:], t["bk"].ap().rearrange("(ft p) one -> p (ft one)", p=P))
    nc.sync.dma_start(bq_sb[:], t["bq"].ap().rearrange("(ft p) one -> p (ft one)", p=P))
    def load_nm_quad(nmt, qq, quad):
        nc.sync.dma_start(
            nmt[:].rearrange("p (ks q) -> p ks q", ks=KS)[:, quad * 4:(quad + 1) * 4, :],
            t["nmT"].ap().rearrange("(ks p) q -> p ks q", p=P)[
                :, quad * 4:(quad + 1) * 4, qq * QW:(qq + 1) * QW],
        )

    load_x_chunk(xk_sb, "xkT", 1)
    load_x_chunk(xk_sb, "xkT", 2)
    load_x_chunk(xk_sb, "xkT", 3)
    load_w(wv_sb, "wvT", DS)
    nm00 = nm_pool.tile([P, KS * QW], bf16, tag="nm", name="nm0_0")
    nm_tiles[0] = nm00
    load_x_chunk(xv_sb, "xvT", 0)
    load_x_chunk(xq_sb, "xqT", 1)
    load_x_chunk(xv_sb, "xvT", 1)
    load_nm_quad(nm00, 0, 0)
    load_nm_quad(nm00, 0, 1)
    load_x_chunk(xv_sb, "xvT", 2)
    load_nm_quad(nm00, 0, 2)
    load_x_chunk(xv_sb, "xvT", 3)
    load_nm_quad(nm00, 0, 3)
    load_nm(0, 1)
    nc.sync.dma_start(ident_sb[:], t["ident"].ap())
    # nm(0,2)/nm(0,3) park on their buffers' WAR (earlier mask reads), but
    # everything queued behind them here is needed later, so parking on the
    # SP queue is harmless
    load_x_chunk(xq_sb, "xqT", 2)
    load_nm(0, 2)
    load_x_chunk(xq_sb, "xqT", 3)
    load_nm(0, 3)
    load_w(wom_sb, "womT", 2)

    # ---- projection work units, split small so the weave never blocks the
    # score/exp stream for long (PE executes in emission order) ----
    qk_state = {}

    def qk_half(kind, ft, tc4, half):
        w_sb, x_sb = (wk_sb, xk_sb) if kind == "k" else (wq_sb, xq_sb)
        if half == 0:
            ps = pj_pool.tile([P, QW], f32, tag="pj", name=f"pj_{kind}{ft}_{tc4}")
            qk_state[(kind, ft, tc4)] = ps
        else:
            ps = qk_state.pop((kind, ft, tc4))
        for ds in range(half * 4, half * 4 + 4):
            nc.tensor.matmul(
                ps[:],
                w_sb[:, ds * F + ft * P: ds * F + (ft + 1) * P],
                x_sb[:, ds * S + tc4 * QW: ds * S + (tc4 + 1) * QW],
                start=(ds == 0),
                stop=(ds == DS - 1),
            )
        if half == 1:
            out_tiles, b_sb = (kT, bk_sb) if kind == "k" else (qT, bq_sb)
            nc.vector.tensor_scalar_add(
                out_tiles[ft][:, tc4 * QW:(tc4 + 1) * QW], ps[:], b_sb[:, ft:ft + 1]
            )

    def vp_tt(hp, tt):
        # v projection for head pair hp, token slice tt
        ps = pj_pool.tile([P, QW], f32, tag="pj")
        for ds in range(DS):
            nc.tensor.matmul(
                ps[:, 0:P],
                xv_sb[:, ds * S + tt * P: ds * S + (tt + 1) * P],
                wv_sb[:, ds * F + hp * P: ds * F + (hp + 1) * P],
                start=(ds == 0),
                stop=(ds == DS - 1),
            )
        nc.vector.tensor_copy(
            v_view[:, tt, 2 * hp:2 * hp + 2, 0:DH],
            ps[:, 0:P].rearrange("p (h f) -> p h f", h=2),
        )

    # ---- foreign-work weave with deadline forcing ----
    foreign = []
    done = set()

    def _run(key):
        if key[0] == "v":
            vp_tt(key[1], key[2])
        else:
            qk_half(key[0], key[1], key[2], key[3])
        done.add(key)

    def pump(n=1):
        while n > 0 and foreign:
            _run(foreign.pop(0))
            n -= 1

    def require(key):
        while key not in done:
            assert foreign, f"foreign queue empty but {key} missing"
            _run(foreign.pop(0))

    # prologue compute: first key chunk + first query quarter of ft0
    for half in range(2):
        qk_half("k", 0, 0, half)
    for half in range(2):
        qk_half("q", 0, 0, half)
    done.add(("k", 0, 0, 1))
    done.add(("q", 0, 0, 1))

    def _qk_keys(kind, ft, tc4):
        return [(kind, ft, tc4, 0), (kind, ft, tc4, 1)]

    # ordered by DMA readiness and consumption deadline
    foreign.extend(
        _qk_keys("k", 0, 1) + _qk_keys("k", 0, 2) + _qk_keys("k", 0, 3)
        + [("v", 0, tt) for tt in range(KS)]
        + _qk_keys("q", 0, 1)
        + _qk_keys("k", 1, 0) + _qk_keys("k", 1, 1)
        + _qk_keys("k", 1, 2) + _qk_keys("k", 1, 3)
        + _qk_keys("q", 0, 2) + _qk_keys("q", 1, 0) + _qk_keys("q", 1, 1)
        + _qk_keys("q", 0, 3)
        + [("v", 1, tt) for tt in range(KS)]
        + _qk_keys("q", 1, 2) + _qk_keys("q", 1, 3)
    )

    # ---- output-projection work units (woven after each quarter's last norm)
    ctxT = {}       # (qq, fs) -> tile
    cn_tiles = {}   # (h, qq) -> [4 normalized ctx blocks in [q, f] layout]
    opq = []        # pending transpose / output-projection units
    op_pool = None

    def transp_unit(h, qq):
        ft, ro = h // 2, (h % 2) * DH
        if (qq, ft) not in ctxT:
            ctxT[(qq, ft)] = ct_pool.tile([P, QW], bf16, tag=f"ct{ft}", name=f"ct{ft}_{qq}")
        trt = tr_pool.tile([DH, 4 * 2 * P], bf16, tag="tr")
        trv = trt[:].rearrange("p (b w) -> p b w", b=4)
        for qb in range(4):
            nc.tensor.matmul(
                trv[:, qb, 0:P], cn_tiles[(h, qq)][qb][:], ident_sb[:],
                is_transpose=True,
            )
        nc.vector.tensor_copy(
            ctxT[(qq, ft)][ro:ro + DH, :].rearrange("p (b w) -> p b w", b=4),
            trv[:, :, 0:P],
        )

    def op_unit(qq, ot):
        ps = op_pool.tile([P, QW], f32, tag="op")
        for fs in range(2):
            nc.tensor.matmul(
                ps[:],
                wom_sb[:, fs * D + ot * P: fs * D + (ot + 1) * P],
                ctxT[(qq, fs)][:],
                start=(fs == 0),
                stop=(fs == 1),
            )
        dst = t["outT"].ap()[ot * P:(ot + 1) * P, qq * QW:(qq + 1) * QW]
        ob = ob_pool.tile([P, QW], bf16, tag="ob")
        if qq == NQ - 1 and ot % 2 == 1:
            nc.scalar.copy(ob[:], ps[:])   # tail: ACT is idle by then
        else:
            nc.vector.tensor_copy(ob[:], ps[:])
        nc.sync.dma_start(dst, ob[:])

    def run_opq(n):
        while n > 0 and opq:
            kind, h_or_ot, qq = opq.pop(0)
            if kind == "tr":
                transp_unit(h_or_ot, qq)
            else:
                op_unit(qq, h_or_ot)
            n -= 1

    # ---- session machinery: scores/exp/mask for (h, qq); the PREVIOUS
    # session's ctx matmuls and normalization weave into this session's slots
    # so the exp stream never waits on ctx inputs.
    def ctx_qb(st, qb):
        # one q-block's FULL 16-step accumulation as a consecutive run: a
        # PSUM bank supports only one open accumulation group at a time, so
        # groups sharing the cx bank must never interleave
        h = st["h"]
        for tt in range(KS):
            require(("v", h // 2, tt))
        for ks in range(KS):
            pt = st["pts"][ks // 2]
            j = ks % 2
            nc.tensor.matmul(
                st["cxv"][:, qb, 0:VW],
                pt[:, j * QW + qb * P: j * QW + (qb + 1) * P],
                v_view[:, ks, h, :],
                start=(ks == 0),
                stop=(ks == KS - 1),
            )

    tr_open = [False]
    tr_pushed = set()

    def norm(st):
        h, qq, cxv = st["h"], st["qq"], st["cxv"]
        blocks = []
        for qb in range(4):
            rs = rs_pool.tile([P, 1], f32, tag="rs")
            nc.vector.reciprocal(rs[:], cxv[:, qb, DH:DH + 1])
            cn_t = cn_pool.tile([P, DH], bf16, tag="cn", name=f"cn{h}_{qq}_{qb}")
            nc.vector.tensor_scalar_mul(cn_t[:], cxv[:, qb, 0:DH], rs[:])
            if dbg is not None and h == 0:
                nc.sync.dma_start(
                    dbg["cn"].ap()[(qq * 4 + qb) * P:(qq * 4 + qb + 1) * P, :], cn_t[:]
                )
            blocks.append(cn_t)
        cn_tiles[(h, qq)] = blocks
        if tr_open[0]:
            # queue transposes for every normalized head of this quarter
            for hh in range(4):
                if (hh, qq) in cn_tiles and (hh, qq) not in tr_pushed:
                    tr_pushed.add((hh, qq))
                    opq.append(("tr", hh, qq))
        if h == 3:
            for ot in range(8):
                opq.append(("op", ot, qq))

    def run_session(h, qq, prev, quota, ctx_start=0, ctx_end=6, self_ctx=False):
        ft, ro = h // 2, (h % 2) * DH
        require(("q", ft, qq, 1))
        # 4 q-block accumulation regions, each 512B-aligned within one bank
        cx = cx_pool.tile([P, 4 * P], f32, tag="cx")
        st = {
            "h": h, "qq": qq, "pts": [None] * NPAIR,
            "cxv": cx[:].rearrange("p (b w) -> p b w", b=4),
        }
        nmv = nm_tiles[qq][:].rearrange("p (ks q) -> p ks q", ks=KS)
        ctx_slots = [[] for _ in range(NPAIR)]
        if prev is not None:
            nsl = ctx_end - ctx_start
            for qb in range(4):
                ctx_slots[ctx_start + (qb * nsl) // 4].append(qb)
        pumped = 0
        for p in range(NPAIR):
            require(("k", ft, p // 2, 1))
            ps = sc_pool.tile([P, 2 * QW], f32, tag="sc")
            for j in (0, 1):
                ks = 2 * p + j
                nc.tensor.matmul(
                    ps[:, j * QW:(j + 1) * QW],
                    kT[ft][ro:ro + DH, ks * P:(ks + 1) * P],
                    qT[ft][ro:ro + DH, qq * QW:(qq + 1) * QW],
                    start=True,
                    stop=True,
                )
            pt = pt_pool.tile([P, 2 * QW], bf16, tag="pt")
            nc.scalar.activation(pt[:], ps[:], Exp, scale=0.125)
            ptv = pt[:].rearrange("p (j q) -> p j q", j=2)
            nc.vector.tensor_mul(ptv, ptv, nmv[:, 2 * p:2 * p + 2, :])
            st["pts"][p] = pt
            if dbg is not None and h == 0:
                nc.sync.dma_start(
                    dbg["pT0"].ap().rearrange(
                        "(pp two pr) q -> pr pp two q", pr=P, two=2)
                    [:, p, :, qq * QW:(qq + 1) * QW],
                    pt[:].rearrange("p (two q) -> p two q", two=2),
                )
            if prev is not None:
                # keep the v-projection weave paced even before its q-block
                # chunk needs it
                require(("v", prev["h"] // 2, 2 * p))
                require(("v", prev["h"] // 2, 2 * p + 1))
                for qb in ctx_slots[p]:
                    ctx_qb(prev, qb)
                if p == ctx_end:
                    norm(prev)
            if opq:
                run_opq(2)
            elif pumped < quota:
                pump(1)
                pumped += 1
        if prev is not None and ctx_end >= NPAIR:
            norm(prev)
        if self_ctx:
            for qb in range(4):
                ctx_qb(st, qb)
            norm(st)
        return st

    # ---- main schedule: 16 sessions, software-pipelined ----
    order = [(h0 + dh, qq) for ph, h0 in ((1, 0), (2, 2)) for qq in range(NQ)
             for dh in (0, 1)]
    prev = None
    op_cm = tr_cm = None
    for si, (h, qq) in enumerate(order):
        # mask prefetch for phase 2 (phase-1 masks are all in the prologue)
        if (h, qq) == (1, 2):
            load_nm(1, 0)
        elif (h, qq) == (3, 0):
            load_nm(1, 1)
        elif (h, qq) == (3, 1):
            load_nm(1, 2)
        elif (h, qq) == (3, 2):
            load_nm(1, 3)
        if si == 10:
            # all projection work must be done now; swap pj's PSUM banks to
            # the transpose + output-projection pools
            while foreign:
                pump(1)
            pj_cm.__exit__(None, None, None)
            tr_cm = tc.tile_pool(name="tr", bufs=1, space="PSUM")
            tr_pool = tr_cm.__enter__()
            op_cm = tc.tile_pool(name="op", bufs=1, space="PSUM")
            op_pool = op_cm.__enter__()
            tr_open[0] = True
        prev = run_session(h, qq, prev,
                           quota=0 if si <= 1 else (8 if si == 9 else 4),
                           ctx_start=4 if si == 1 else 0,
                           ctx_end=8 if si == 1 else 6,
                           self_ctx=(si == 15))

    # tail: last quarter's transposes, then a double-buffered outproj burst
    while opq and opq[0][0] == "tr":
        kind, hh, qq = opq.pop(0)
        transp_unit(hh, qq)
    op_cm.__exit__(None, None, None)
    tr_cm.__exit__(None, None, None)
    cx_cm.__exit__(None, None, None)
    op_cm = tc.tile_pool(name="op2", bufs=4, space="PSUM")
    op_pool = op_cm.__enter__()
    run_opq(len(opq))

    if dbg is not None:
        for ft in range(2):
            nc.sync.dma_start(dbg["qT"].ap()[ft * P:(ft + 1) * P, :], qT[ft][:])
            nc.sync.dma_start(dbg["kT"].ap()[ft * P:(ft + 1) * P, :], kT[ft][:])
        nc.sync.dma_start(dbg["v"].ap(), v_sb[:])
        for qq in range(NQ):
            for fs in range(2):
                nc.sync.dma_start(
                    dbg["ctxT"].ap()[fs * P:(fs + 1) * P, qq * QW:(qq + 1) * QW],
                    ctxT[(qq, fs)][:],
                )

    assert not foreign and not opq, f"undrained work: {foreign} {opq}"
    op_cm.__exit__(None, None, None)
    es.close()


@functools.lru_cache(maxsize=1)
def _build(debug=False):
    import concourse.bacc as bacc
    import concourse.mybir as mybir
    import concourse.tile as tile

    bf16 = mybir.dt.bfloat16
    f32 = mybir.dt.float32

    nc = bacc.Bacc("TRN2", target_bir_lowering=False, debug=False, num_devices=NCORES)
    t = {
        "xqT": nc.dram_tensor("xqT", (D, S), bf16, kind="ExternalInput"),
        "xkT": nc.dram_tensor("xkT", (D, S), bf16, kind="ExternalInput"),
        "xvT": nc.dram_tensor("xvT", (D, S), bf16, kind="ExternalInput"),
        "wqT": nc.dram_tensor("wqT", (D, F), bf16, kind="ExternalInput"),
        "wkT": nc.dram_tensor("wkT", (D, F), bf16, kind="ExternalInput"),
        "wvT": nc.dram_tensor("wvT", (D, F), bf16, kind="ExternalInput"),
        "womT": nc.dram_tensor("womT", (F, D), bf16, kind="ExternalInput"),
        "nmT": nc.dram_tensor("nmT", (S, S), bf16, kind="ExternalInput"),
        "bq": nc.dram_tensor("bq", (F, 1), f32, kind="ExternalInput"),
        "bk": nc.dram_tensor("bk", (F, 1), f32, kind="ExternalInput"),
        "ident": nc.dram_tensor("ident", (P, P), bf16, kind="ExternalInput"),
        "outT": nc.dram_tensor("outT", (D, S), bf16, kind="ExternalOutput"),
    }
    dbg = None
    if debug:
        dbg = {
            "qT": nc.dram_tensor("dbg_qT", (F, S), bf16, kind="ExternalOutput"),
            "kT": nc.dram_tensor("dbg_kT", (F, S), bf16, kind="ExternalOutput"),
            "v": nc.dram_tensor("dbg_v", (P, KS * NH * VW), bf16, kind="ExternalOutput"),
            "pT0": nc.dram_tensor("dbg_pT0", (S, S), bf16, kind="ExternalOutput"),
            "ctxT": nc.dram_tensor("dbg_ctxT", (F, S), bf16, kind="ExternalOutput"),
            "cn": nc.dram_tensor("dbg_cn", (S, DH), bf16, kind="ExternalOutput"),
        }
    with tile.TileContext(nc) as tc:
        _emit(nc, tc, t, dbg)
    nc.compile()
    return nc


def _prep_core_inputs(c, Q, K, V, mask, Wq, bq, Wk, bk, Wv, Wo, _cache={}):
    import ml_dtypes

    bf = ml_dtypes.bfloat16
    b, g = divmod(c, GROUPS)
    bkey = ("batch", b, id(Q))
    if bkey not in _cache:
        _cache.clear()
        for bb in range(B):
            nm = 1.0 - mask[bb, 0].astype(np.float32)
            _cache[("batch", bb, id(Q))] = {
                "xqT": Q[bb].T.astype(bf),
                "xkT": K[bb].T.astype(bf),
                "xvT": V[bb].T.astype(bf),
                "nmT": nm.T.astype(bf),
            }
    fsl = slice(g * F, (g + 1) * F)
    return {
        **_cache[bkey],
        "wqT": Wq[fsl, :].T.astype(bf),
        "wkT": Wk[fsl, :].T.astype(bf),
        "wvT": Wv[fsl, :].T.astype(bf),
        "womT": Wo[:, fsl].T.astype(bf),
        "bq": bq[fsl].reshape(F, 1).astype(np.float32),
        "bk": bk[fsl].reshape(F, 1).astype(np.float32),
        "ident": np.eye(P, dtype=bf),
    }


def kernel(Q, K, V, mask, Wq, bq, Wk, bk, Wv, bv, Wo, bo, _trace=False, _tmpdir=None):
    from concourse.bass_utils import run_bass_kernel_spmd

    Q, K, V = np.asarray(Q, np.float32), np.asarray(K, np.float32), np.asarray(V, np.float32)
    mask = np.asarray(mask)
    Wq, Wk, Wv, Wo = (np.asarray(w, np.float32) for w in (Wq, Wk, Wv, Wo))
    bq, bk, bv, bo = (np.asarray(x, np.float32) for x in (bq, bk, bv, bo))

    nc = _build()
    in_maps = [_prep_core_inputs(c, Q, K, V, mask, Wq, bq, Wk, bk, Wv, Wo) for c in range(NCORES)]
    kw = {}
    if _trace:
        kw = dict(trace=True, tmpdir=_tmpdir)
    res = run_bass_kernel_spmd(nc, in_maps, core_ids=list(range(NCORES)), **kw)

    const = (Wo @ bv + bo).astype(np.float32)  # softmax rows sum to 1 -> bv enters linearly
    out = np.empty((B, S, D), np.float32)
    for b in range(B):
        acc = res.results[b * GROUPS]["outT"].astype(np.float32)
        for g in range(1, GROUPS):
            acc = acc + res.results[b * GROUPS + g]["outT"].astype(np.float32)
        out[b] = acc.T + const
    if _trace:
        kernel._last_results = res
    return out


# revision 78
# speedup vs baseline: 1.1808x; 1.0115x over previous
"""Multi-head attention (B=2, S=2048, D=1024, H=16) on 8 trn2 NeuronCores.

Sharding: data-parallel over batch (2) x tensor-parallel over head-groups (4).
Core c handles batch c//4, heads [4*(c%4), 4*(c%4)+4).  Each core computes
q/k/v projections for its 256 head-features, masked softmax attention for its
4 heads, and the row-shard of the output projection; partial outputs are
summed on the host during the gather step.

Schedule: attention is blocked into 4 query-quarters of 512.  The ctx matmul
runs with pt as the stationary operand (q on the output partitions, head
features on the free dim), which halves its PE cost versus the v-stationary
orientation; the softmax denominator rides along as a 65th v column and is
divided out with a per-partition reciprocal before a PE transpose puts ctx
back into [feature, q] layout for the output projection.  Projections for
later head groups are woven between attention score pairs so the activation
engine (exp) starts early and stays fed.
"""

import sys
import functools
from contextlib import ExitStack

sys.path.insert(0, "/opt/trn_rl_repo")

import numpy as np

B, S, D, H = 2, 2048, 1024, 16
DH = 64
P = 128
NCORES = 8
GROUPS = 4            # head groups = cores per batch
NH = H // GROUPS      # heads per core = 4
F = NH * DH           # head features per core = 256
KS = S // P           # 16 key slices
DS = D // P           # 8 d_model slices
QW = 512              # query quarter width
NQ = S // QW          # 4 quarters
NPAIR = KS // 2       # 8 key-slice pairs
VW = DH + 1           # v width per head incl. ones column = 65
DEPTH = 4             # ctx lag behind scores, in key-slice pairs


def _emit(nc, tc, t, dbg=None):
    import concourse.mybir as mybir
    bf16 = mybir.dt.bfloat16
    f32 = mybir.dt.float32
    Exp = mybir.ActivationFunctionType.Exp

    es = ExitStack()
    ep = lambda cm: es.enter_context(cm)
    const_pool = ep(tc.tile_pool(name="const", bufs=1))
    w_pool = ep(tc.tile_pool(name="w", bufs=1))
    x_pool = ep(tc.tile_pool(name="x", bufs=1))
    qk_pool = ep(tc.tile_pool(name="qk", bufs=1))
    v_pool = ep(tc.tile_pool(name="v", bufs=1))
    nm_pool = ep(tc.tile_pool(name="nm", bufs=2))
    pt_pool = ep(tc.tile_pool(name="pt", bufs=13))
    cn_pool = ep(tc.tile_pool(name="cn", bufs=40))
    rs_pool = ep(tc.tile_pool(name="rs", bufs=2))
    ct_pool = ep(tc.tile_pool(name="ct", bufs=2))
    ob_pool = ep(tc.tile_pool(name="ob", bufs=4))
    # PSUM banks: sc 2x2 + cx 2x1 + pj 2x1 = 8 during the projection phase;
    # pj's two banks are handed to tr (1) + op (1) once projections finish.
    # pj is opened last so its mid-emit release keeps the pool stack LIFO.
    sc_pool = ep(tc.tile_pool(name="sc", bufs=2, space="PSUM"))
    cx_cm = tc.tile_pool(name="cx", bufs=2, space="PSUM")
    cx_pool = cx_cm.__enter__()
    tr_pool = None
    pj_cm = tc.tile_pool(name="pj", bufs=2, space="PSUM")
    pj_pool = pj_cm.__enter__()

    # ---- SBUF tiles ----
    bq_sb = const_pool.tile([P, 2], f32, tag="bq")
    bk_sb = const_pool.tile([P, 2], f32, tag="bk")
    ident_sb = const_pool.tile([P, P], bf16, tag="ident")
    wu_sb = const_pool.tile([P, 128], bf16, tag="wu")  # warmup garbage

    wq_sb = w_pool.tile([P, DS * F], bf16, tag="wq")
    wk_sb = w_pool.tile([P, DS * F], bf16, tag="wk")
    wv_sb = w_pool.tile([P, DS * F], bf16, tag="wv")
    wom_sb = w_pool.tile([P, 2 * D], bf16, tag="wom")

    xk_sb = x_pool.tile([P, DS * S], bf16, tag="xk")
    xq_sb = x_pool.tile([P, DS * S], bf16, tag="xq")
    xv_sb = x_pool.tile([P, DS * S], bf16, tag="xv")

    qT = [qk_pool.tile([P, S], bf16, tag=f"qT{ft}", name=f"qT{ft}") for ft in range(2)]
    kT = [qk_pool.tile([P, S], bf16, tag=f"kT{ft}", name=f"kT{ft}") for ft in range(2)]

    v_sb = v_pool.tile([P, KS * NH * VW], bf16, tag="v")
    v_view = v_sb[:].rearrange("p (ks h w) -> p ks h w", ks=KS, h=NH)
    nc.gpsimd.memset(v_sb[:], 1.0)

    # ---- PE warmup: contentless matmuls to climb the p-state ramp while the
    # first input DMAs are in flight. Results are discarded.
    nc.gpsimd.memset(wu_sb[:], 0.0)
    for wu in range(20):
        ps = pj_pool.tile([P, QW], f32, tag="pj", name=f"wu{wu}")
        nc.tensor.matmul(ps[:, 0:P], wu_sb[:, 0:P], wu_sb[:], start=True, stop=True)

    # ---- DMA prologue (SP queue) ----
    def load_w(w_sb, wname, ng):
        nc.sync.dma_start(
            w_sb[:].rearrange("p (g f) -> p g f", g=ng),
            t[wname].ap().rearrange("(g p) f -> p g f", p=P),
        )

    def load_x_chunk(x_sb, xname, c):
        nc.sync.dma_start(
            x_sb[:].rearrange("p (ds q) -> p ds q", ds=DS)[:, :, c * QW:(c + 1) * QW],
            t[xname].ap().rearrange("(ds p) q -> p ds q", p=P)[:, :, c * QW:(c + 1) * QW],
        )

    nm_tiles = {}

    def load_nm(ph, qq):
        # mask slice for quarter qq: [P, KS, QW], four DMAs of 4 key slices
        # each so early score pairs unblock as soon as their slice lands
        nmt = nm_pool.tile([P, KS * QW], bf16, tag="nm", name=f"nm{ph}_{qq}")
        for quad in range(4):
            nc.sync.dma_start(
                nmt[:].rearrange("p (ks q) -> p ks q", ks=KS)[:, quad * 4:(quad + 1) * 4, :],
                t["nmT"].ap().rearrange("(ks p) q -> p ks q", p=P)[
                    :, quad * 4:(quad + 1) * 4, qq * QW:(qq + 1) * QW],
            )
        nm_tiles[qq] = nmt

    # Ordered by first consumption on the serial DMA device.  The exp stream
    # only needs wk/xk + wq/xq-quarter0; the mask multiply (DVE) and the ctx
    # weave (one session behind) tolerate later arrival, so nm and xv follow
    # the full xk.
    load_w(wk_sb, "wkT", DS)
    load_x_chunk(xk_sb, "xkT", 0)
    load_w(wq_sb, "wqT", DS)
    load_x_chunk(xq_sb, "xqT", 0)
    nc.sync.dma_start(bk_sb[:], t["bk"].ap().rearrange("(ft p) one -> p (ft one)", p=P))
    nc.sync.dma_start(bq_sb[:], t["bq"].ap().rearrange("(ft p) one -> p (ft one)", p=P))
    def load_nm_quad(nmt, qq, quad):
        nc.sync.dma_start(
            nmt[:].rearrange("p (ks q) -> p ks q", ks=KS)[:, quad * 4:(quad + 1) * 4, :],
            t["nmT"].ap().rearrange("(ks p) q -> p ks q", p=P)[
                :, quad * 4:(quad + 1) * 4, qq * QW:(qq + 1) * QW],
        )

    load_x_chunk(xk_sb, "xkT", 1)
    load_x_chunk(xk_sb, "xkT", 2)
    load_x_chunk(xk_sb, "xkT", 3)
    load_w(wv_sb, "wvT", DS)
    nm00 = nm_pool.tile([P, KS * QW], bf16, tag="nm", name="nm0_0")
    nm_tiles[0] = nm00
    load_x_chunk(xv_sb, "xvT", 0)
    load_x_chunk(xq_sb, "xqT", 1)
    load_x_chunk(xv_sb, "xvT", 1)
    load_nm_quad(nm00, 0, 0)
    load_nm_quad(nm00, 0, 1)
    load_x_chunk(xv_sb, "xvT", 2)
    load_nm_quad(nm00, 0, 2)
    load_x_chunk(xv_sb, "xvT", 3)
    load_nm_quad(nm00, 0, 3)
    load_nm(0, 1)
    nc.sync.dma_start(ident_sb[:], t["ident"].ap())
    # nm(0,2)/nm(0,3) park on their buffers' WAR (earlier mask reads), but
    # everything queued behind them here is needed later, so parking on the
    # SP queue is harmless
    load_x_chunk(xq_sb, "xqT", 2)
    load_nm(0, 2)
    load_x_chunk(xq_sb, "xqT", 3)
    load_nm(0, 3)
    load_w(wom_sb, "womT", 2)

    # ---- projection work units, split small so the weave never blocks the
    # score/exp stream for long (PE executes in emission order) ----
    qk_state = {}

    def qk_half(kind, ft, tc4, half):
        w_sb, x_sb = (wk_sb, xk_sb) if kind == "k" else (wq_sb, xq_sb)
        if half == 0:
            ps = pj_pool.tile([P, QW], f32, tag="pj", name=f"pj_{kind}{ft}_{tc4}")
            qk_state[(kind, ft, tc4)] = ps
        else:
            ps = qk_state.pop((kind, ft, tc4))
        for ds in range(half * 4, half * 4 + 4):
            nc.tensor.matmul(
                ps[:],
                w_sb[:, ds * F + ft * P: ds * F + (ft + 1) * P],
                x_sb[:, ds * S + tc4 * QW: ds * S + (tc4 + 1) * QW],
                start=(ds == 0),
                stop=(ds == DS - 1),
            )
        if half == 1:
            out_tiles, b_sb = (kT, bk_sb) if kind == "k" else (qT, bq_sb)
            nc.vector.tensor_scalar_add(
                out_tiles[ft][:, tc4 * QW:(tc4 + 1) * QW], ps[:], b_sb[:, ft:ft + 1]
            )

    def vp_tt(hp, tt):
        # v projection for head pair hp, token slice tt
        ps = pj_pool.tile([P, QW], f32, tag="pj")
        for ds in range(DS):
            nc.tensor.matmul(
                ps[:, 0:P],
                xv_sb[:, ds * S + tt * P: ds * S + (tt + 1) * P],
                wv_sb[:, ds * F + hp * P: ds * F + (hp + 1) * P],
                start=(ds == 0),
                stop=(ds == DS - 1),
            )
        nc.vector.tensor_copy(
            v_view[:, tt, 2 * hp:2 * hp + 2, 0:DH],
            ps[:, 0:P].rearrange("p (h f) -> p h f", h=2),
        )

    # ---- foreign-work weave with deadline forcing ----
    foreign = []
    done = set()

    def _run(key):
        if key[0] == "v":
            vp_tt(key[1], key[2])
        else:
            qk_half(key[0], key[1], key[2], key[3])
        done.add(key)

    def pump(n=1):
        while n > 0 and foreign:
            _run(foreign.pop(0))
            n -= 1

    def require(key):
        while key not in done:
            assert foreign, f"foreign queue empty but {key} missing"
            _run(foreign.pop(0))

    # prologue compute: first key chunk + first query quarter of ft0
    for half in range(2):
        qk_half("k", 0, 0, half)
    for half in range(2):
        qk_half("q", 0, 0, half)
    done.add(("k", 0, 0, 1))
    done.add(("q", 0, 0, 1))

    def _qk_keys(kind, ft, tc4):
        return [(kind, ft, tc4, 0), (kind, ft, tc4, 1)]

    # ordered by DMA readiness and consumption deadline
    foreign.extend(
        _qk_keys("k", 0, 1) + _qk_keys("k", 0, 2) + _qk_keys("k", 0, 3)
        + [("v", 0, tt) for tt in range(KS)]
        + _qk_keys("q", 0, 1)
        + _qk_keys("k", 1, 0) + _qk_keys("k", 1, 1)
        + _qk_keys("k", 1, 2) + _qk_keys("k", 1, 3)
        + _qk_keys("q", 0, 2) + _qk_keys("q", 1, 0) + _qk_keys("q", 1, 1)
        + _qk_keys("q", 0, 3)
        + [("v", 1, tt) for tt in range(KS)]
        + _qk_keys("q", 1, 2) + _qk_keys("q", 1, 3)
    )

    # ---- output-projection work units (woven after each quarter's last norm)
    ctxT = {}       # (qq, fs) -> tile
    cn_tiles = {}   # (h, qq) -> [4 normalized ctx blocks in [q, f] layout]
    opq = []        # pending transpose / output-projection units
    op_pool = None

    def transp_unit(h, qq):
        ft, ro = h // 2, (h % 2) * DH
        if (qq, ft) not in ctxT:
            ctxT[(qq, ft)] = ct_pool.tile([P, QW], bf16, tag=f"ct{ft}", name=f"ct{ft}_{qq}")
        trt = tr_pool.tile([DH, 4 * 2 * P], bf16, tag="tr")
        trv = trt[:].rearrange("p (b w) -> p b w", b=4)
        for qb in range(4):
            nc.tensor.matmul(
                trv[:, qb, 0:P], cn_tiles[(h, qq)][qb][:], ident_sb[:],
                is_transpose=True,
            )
        nc.vector.tensor_copy(
            ctxT[(qq, ft)][ro:ro + DH, :].rearrange("p (b w) -> p b w", b=4),
            trv[:, :, 0:P],
        )

    def op_unit(qq, ot):
        ps = op_pool.tile([P, QW], f32, tag="op")
        for fs in range(2):
            nc.tensor.matmul(
                ps[:],
                wom_sb[:, fs * D + ot * P: fs * D + (ot + 1) * P],
                ctxT[(qq, fs)][:],
                start=(fs == 0),
                stop=(fs == 1),
            )
        dst = t["outT"].ap()[ot * P:(ot + 1) * P, qq * QW:(qq + 1) * QW]
        ob = ob_pool.tile([P, QW], bf16, tag="ob")
        if qq == NQ - 1 and ot % 2 == 1:
            nc.scalar.copy(ob[:], ps[:])   # tail: ACT is idle by then
        else:
            nc.vector.tensor_copy(ob[:], ps[:])
        nc.sync.dma_start(dst, ob[:])

    def run_opq(n):
        while n > 0 and opq:
            kind, h_or_ot, qq = opq.pop(0)
            if kind == "tr":
                transp_unit(h_or_ot, qq)
            else:
                op_unit(qq, h_or_ot)
            n -= 1

    # ---- session machinery: scores/exp/mask for (h, qq); the PREVIOUS
    # session's ctx matmuls and normalization weave into this session's slots
    # so the exp stream never waits on ctx inputs.
    def ctx_qb(st, qb):
        # one q-block's FULL 16-step accumulation as a consecutive run: a
        # PSUM bank supports only one open accumulation group at a time, so
        # groups sharing the cx bank must never interleave
        h = st["h"]
        for tt in range(KS):
            require(("v", h // 2, tt))
        for ks in range(KS):
            pt = st["pts"][ks // 2]
            j = ks % 2
            nc.tensor.matmul(
                st["cxv"][:, qb, 0:VW],
                pt[:, j * QW + qb * P: j * QW + (qb + 1) * P],
                v_view[:, ks, h, :],
                start=(ks == 0),
                stop=(ks == KS - 1),
            )

    tr_open = [False]
    tr_pushed = set()

    def norm(st):
        h, qq, cxv = st["h"], st["qq"], st["cxv"]
        blocks = []
        for qb in range(4):
            rs = rs_pool.tile([P, 1], f32, tag="rs")
            nc.vector.reciprocal(rs[:], cxv[:, qb, DH:DH + 1])
            cn_t = cn_pool.tile([P, DH], bf16, tag="cn", name=f"cn{h}_{qq}_{qb}")
            nc.vector.tensor_scalar_mul(cn_t[:], cxv[:, qb, 0:DH], rs[:])
            if dbg is not None and h == 0:
                nc.sync.dma_start(
                    dbg["cn"].ap()[(qq * 4 + qb) * P:(qq * 4 + qb + 1) * P, :], cn_t[:]
                )
            blocks.append(cn_t)
        cn_tiles[(h, qq)] = blocks
        if tr_open[0]:
            # queue transposes for every normalized head of this quarter
            for hh in range(4):
                if (hh, qq) in cn_tiles and (hh, qq) not in tr_pushed:
                    tr_pushed.add((hh, qq))
                    opq.append(("tr", hh, qq))
        if h == 3:
            for ot in range(8):
                opq.append(("op", ot, qq))

    def run_session(h, qq, prev, quota, ctx_start=0, ctx_end=6, self_ctx=False):
        ft, ro = h // 2, (h % 2) * DH
        require(("q", ft, qq, 1))
        # 4 q-block accumulation regions, each 512B-aligned within one bank
        cx = cx_pool.tile([P, 4 * P], f32, tag="cx")
        st = {
            "h": h, "qq": qq, "pts": [None] * NPAIR,
            "cxv": cx[:].rearrange("p (b w) -> p b w", b=4),
        }
        nmv = nm_tiles[qq][:].rearrange("p (ks q) -> p ks q", ks=KS)
        ctx_slots = [[] for _ in range(NPAIR)]
        if prev is not None:
            nsl = ctx_end - ctx_start
            for qb in range(4):
                ctx_slots[ctx_start + (qb * nsl) // 4].append(qb)
        pumped = 0
        for p in range(NPAIR):
            require(("k", ft, p // 2, 1))
            ps = sc_pool.tile([P, 2 * QW], f32, tag="sc")
            for j in (0, 1):
                ks = 2 * p + j
                nc.tensor.matmul(
                    ps[:, j * QW:(j + 1) * QW],
                    kT[ft][ro:ro + DH, ks * P:(ks + 1) * P],
                    qT[ft][ro:ro + DH, qq * QW:(qq + 1) * QW],
                    start=True,
                    stop=True,
                )
            pt = pt_pool.tile([P, 2 * QW], bf16, tag="pt")
            nc.scalar.activation(pt[:], ps[:], Exp, scale=0.125)
            ptv = pt[:].rearrange("p (j q) -> p j q", j=2)
            nc.vector.tensor_mul(ptv, ptv, nmv[:, 2 * p:2 * p + 2, :])
            st["pts"][p] = pt
            if dbg is not None and h == 0:
                nc.sync.dma_start(
                    dbg["pT0"].ap().rearrange(
                        "(pp two pr) q -> pr pp two q", pr=P, two=2)
                    [:, p, :, qq * QW:(qq + 1) * QW],
                    pt[:].rearrange("p (two q) -> p two q", two=2),
                )
            if prev is not None:
                # keep the v-projection weave paced even before its q-block
                # chunk needs it
                require(("v", prev["h"] // 2, 2 * p))
                require(("v", prev["h"] // 2, 2 * p + 1))
                for qb in ctx_slots[p]:
                    ctx_qb(prev, qb)
                if p == ctx_end:
                    norm(prev)
            if opq:
                run_opq(2)
            elif pumped < quota:
                pump(1)
                pumped += 1
        if prev is not None and ctx_end >= NPAIR:
            norm(prev)
        if self_ctx:
            for qb in range(4):
                ctx_qb(st, qb)
            norm(st)
        return st

    # ---- main schedule: 16 sessions, software-pipelined ----
    order = [(h0 + dh, qq) for ph, h0 in ((1, 0), (2, 2)) for qq in range(NQ)
             for dh in (0, 1)]
    prev = None
    op_cm = tr_cm = None
    for si, (h, qq) in enumerate(order):
        # mask prefetch for phase 2 (phase-1 masks are all in the prologue)
        if (h, qq) == (1, 2):
            load_nm(1, 0)
        elif (h, qq) == (3, 0):
            load_nm(1, 1)
        elif (h, qq) == (3, 1):
            load_nm(1, 2)
        elif (h, qq) == (3, 2):
            load_nm(1, 3)
        if si == 10:
            # all projection work must be done now; swap pj's PSUM banks to
            # the transpose + output-projection pools
            while foreign:
                pump(1)
            pj_cm.__exit__(None, None, None)
            tr_cm = tc.tile_pool(name="tr", bufs=1, space="PSUM")
            tr_pool = tr_cm.__enter__()
            op_cm = tc.tile_pool(name="op", bufs=1, space="PSUM")
            op_pool = op_cm.__enter__()
            tr_open[0] = True
        prev = run_session(h, qq, prev,
                           quota=0 if si <= 1 else (8 if si == 9 else 4),
                           ctx_start=6 if si == 1 else 0,
                           ctx_end=8 if si == 1 else 6,
                           self_ctx=(si == 15))

    # tail: last quarter's transposes, then a double-buffered outproj burst
    while opq and opq[0][0] == "tr":
        kind, hh, qq = opq.pop(0)
        transp_unit(hh, qq)
    op_cm.__exit__(None, None, None)
    tr_cm.__exit__(None, None, None)
    cx_cm.__exit__(None, None, None)
    op_cm = tc.tile_pool(name="op2", bufs=4, space="PSUM")
    op_pool = op_cm.__enter__()
    run_opq(len(opq))

    if dbg is not None:
        for ft in range(2):
            nc.sync.dma_start(dbg["qT"].ap()[ft * P:(ft + 1) * P, :], qT[ft][:])
            nc.sync.dma_start(dbg["kT"].ap()[ft * P:(ft + 1) * P, :], kT[ft][:])
        nc.sync.dma_start(dbg["v"].ap(), v_sb[:])
        for qq in range(NQ):
            for fs in range(2):
                nc.sync.dma_start(
                    dbg["ctxT"].ap()[fs * P:(fs + 1) * P, qq * QW:(qq + 1) * QW],
                    ctxT[(qq, fs)][:],
                )

    assert not foreign and not opq, f"undrained work: {foreign} {opq}"
    op_cm.__exit__(None, None, None)
    es.close()


@functools.lru_cache(maxsize=1)
def _build(debug=False):
    import concourse.bacc as bacc
    import concourse.mybir as mybir
    import concourse.tile as tile

    bf16 = mybir.dt.bfloat16
    f32 = mybir.dt.float32

    nc = bacc.Bacc("TRN2", target_bir_lowering=False, debug=False, num_devices=NCORES)
    t = {
        "xqT": nc.dram_tensor("xqT", (D, S), bf16, kind="ExternalInput"),
        "xkT": nc.dram_tensor("xkT", (D, S), bf16, kind="ExternalInput"),
        "xvT": nc.dram_tensor("xvT", (D, S), bf16, kind="ExternalInput"),
        "wqT": nc.dram_tensor("wqT", (D, F), bf16, kind="ExternalInput"),
        "wkT": nc.dram_tensor("wkT", (D, F), bf16, kind="ExternalInput"),
        "wvT": nc.dram_tensor("wvT", (D, F), bf16, kind="ExternalInput"),
        "womT": nc.dram_tensor("womT", (F, D), bf16, kind="ExternalInput"),
        "nmT": nc.dram_tensor("nmT", (S, S), bf16, kind="ExternalInput"),
        "bq": nc.dram_tensor("bq", (F, 1), f32, kind="ExternalInput"),
        "bk": nc.dram_tensor("bk", (F, 1), f32, kind="ExternalInput"),
        "ident": nc.dram_tensor("ident", (P, P), bf16, kind="ExternalInput"),
        "outT": nc.dram_tensor("outT", (D, S), bf16, kind="ExternalOutput"),
    }
    dbg = None
    if debug:
        dbg = {
            "qT": nc.dram_tensor("dbg_qT", (F, S), bf16, kind="ExternalOutput"),
            "kT": nc.dram_tensor("dbg_kT", (F, S), bf16, kind="ExternalOutput"),
            "v": nc.dram_tensor("dbg_v", (P, KS * NH * VW), bf16, kind="ExternalOutput"),
            "pT0": nc.dram_tensor("dbg_pT0", (S, S), bf16, kind="ExternalOutput"),
            "ctxT": nc.dram_tensor("dbg_ctxT", (F, S), bf16, kind="ExternalOutput"),
            "cn": nc.dram_tensor("dbg_cn", (S, DH), bf16, kind="ExternalOutput"),
        }
    with tile.TileContext(nc) as tc:
        _emit(nc, tc, t, dbg)
    nc.compile()
    return nc


def _prep_core_inputs(c, Q, K, V, mask, Wq, bq, Wk, bk, Wv, Wo, _cache={}):
    import ml_dtypes

    bf = ml_dtypes.bfloat16
    b, g = divmod(c, GROUPS)
    bkey = ("batch", b, id(Q))
    if bkey not in _cache:
        _cache.clear()
        for bb in range(B):
            nm = 1.0 - mask[bb, 0].astype(np.float32)
            _cache[("batch", bb, id(Q))] = {
                "xqT": Q[bb].T.astype(bf),
                "xkT": K[bb].T.astype(bf),
                "xvT": V[bb].T.astype(bf),
                "nmT": nm.T.astype(bf),
            }
    fsl = slice(g * F, (g + 1) * F)
    return {
        **_cache[bkey],
        "wqT": Wq[fsl, :].T.astype(bf),
        "wkT": Wk[fsl, :].T.astype(bf),
        "wvT": Wv[fsl, :].T.astype(bf),
        "womT": Wo[:, fsl].T.astype(bf),
        "bq": bq[fsl].reshape(F, 1).astype(np.float32),
        "bk": bk[fsl].reshape(F, 1).astype(np.float32),
        "ident": np.eye(P, dtype=bf),
    }


def kernel(Q, K, V, mask, Wq, bq, Wk, bk, Wv, bv, Wo, bo, _trace=False, _tmpdir=None):
    from concourse.bass_utils import run_bass_kernel_spmd

    Q, K, V = np.asarray(Q, np.float32), np.asarray(K, np.float32), np.asarray(V, np.float32)
    mask = np.asarray(mask)
    Wq, Wk, Wv, Wo = (np.asarray(w, np.float32) for w in (Wq, Wk, Wv, Wo))
    bq, bk, bv, bo = (np.asarray(x, np.float32) for x in (bq, bk, bv, bo))

    nc = _build()
    in_maps = [_prep_core_inputs(c, Q, K, V, mask, Wq, bq, Wk, bk, Wv, Wo) for c in range(NCORES)]
    kw = {}
    if _trace:
        kw = dict(trace=True, tmpdir=_tmpdir)
    res = run_bass_kernel_spmd(nc, in_maps, core_ids=list(range(NCORES)), **kw)

    const = (Wo @ bv + bo).astype(np.float32)  # softmax rows sum to 1 -> bv enters linearly
    out = np.empty((B, S, D), np.float32)
    for b in range(B):
        acc = res.results[b * GROUPS]["outT"].astype(np.float32)
        for g in range(1, GROUPS):
            acc = acc + res.results[b * GROUPS + g]["outT"].astype(np.float32)
        out[b] = acc.T + const
    if _trace:
        kernel._last_results = res
    return out


# revision 83
# speedup vs baseline: 1.1908x; 1.0084x over previous
"""Multi-head attention (B=2, S=2048, D=1024, H=16) on 8 trn2 NeuronCores.

Sharding: data-parallel over batch (2) x tensor-parallel over head-groups (4).
Core c handles batch c//4, heads [4*(c%4), 4*(c%4)+4).  Each core computes
q/k/v projections for its 256 head-features, masked softmax attention for its
4 heads, and the row-shard of the output projection; partial outputs are
summed on the host during the gather step.

Schedule: attention is blocked into 4 query-quarters of 512.  The ctx matmul
runs with pt as the stationary operand (q on the output partitions, head
features on the free dim), which halves its PE cost versus the v-stationary
orientation; the softmax denominator rides along as a 65th v column and is
divided out with a per-partition reciprocal before a PE transpose puts ctx
back into [feature, q] layout for the output projection.  Projections for
later head groups are woven between attention score pairs so the activation
engine (exp) starts early and stays fed.
"""

import sys
import functools
from contextlib import ExitStack

sys.path.insert(0, "/opt/trn_rl_repo")

import numpy as np

B, S, D, H = 2, 2048, 1024, 16
DH = 64
P = 128
NCORES = 8
GROUPS = 4            # head groups = cores per batch
NH = H // GROUPS      # heads per core = 4
F = NH * DH           # head features per core = 256
KS = S // P           # 16 key slices
DS = D // P           # 8 d_model slices
QW = 512              # query quarter width
NQ = S // QW          # 4 quarters
NPAIR = KS // 2       # 8 key-slice pairs
VW = DH + 1           # v width per head incl. ones column = 65
DEPTH = 4             # ctx lag behind scores, in key-slice pairs


def _emit(nc, tc, t, dbg=None):
    import concourse.mybir as mybir
    bf16 = mybir.dt.bfloat16
    f32 = mybir.dt.float32
    Exp = mybir.ActivationFunctionType.Exp

    es = ExitStack()
    ep = lambda cm: es.enter_context(cm)
    const_pool = ep(tc.tile_pool(name="const", bufs=1))
    w_pool = ep(tc.tile_pool(name="w", bufs=1))
    x_pool = ep(tc.tile_pool(name="x", bufs=1))
    qk_pool = ep(tc.tile_pool(name="qk", bufs=1))
    v_pool = ep(tc.tile_pool(name="v", bufs=1))
    nm_pool = ep(tc.tile_pool(name="nm", bufs=2))
    pt_pool = ep(tc.tile_pool(name="pt", bufs=13))
    cn_pool = ep(tc.tile_pool(name="cn", bufs=40))
    rs_pool = ep(tc.tile_pool(name="rs", bufs=2))
    ct_pool = ep(tc.tile_pool(name="ct", bufs=2))
    ob_pool = ep(tc.tile_pool(name="ob", bufs=4))
    # PSUM banks: sc 2x2 + cx 2x1 + pj 2x1 = 8 during the projection phase;
    # pj's two banks are handed to tr (1) + op (1) once projections finish.
    # pj is opened last so its mid-emit release keeps the pool stack LIFO.
    sc_pool = ep(tc.tile_pool(name="sc", bufs=2, space="PSUM"))
    cx_cm = tc.tile_pool(name="cx", bufs=2, space="PSUM")
    cx_pool = cx_cm.__enter__()
    tr_pool = None
    pj_cm = tc.tile_pool(name="pj", bufs=2, space="PSUM")
    pj_pool = pj_cm.__enter__()

    # ---- SBUF tiles ----
    bq_sb = const_pool.tile([P, 2], f32, tag="bq")
    bk_sb = const_pool.tile([P, 2], f32, tag="bk")
    ident_sb = const_pool.tile([P, P], bf16, tag="ident")
    wu_sb = const_pool.tile([P, 128], bf16, tag="wu")  # warmup garbage

    wq_sb = w_pool.tile([P, DS * F], bf16, tag="wq")
    wk_sb = w_pool.tile([P, DS * F], bf16, tag="wk")
    wv_sb = w_pool.tile([P, DS * F], bf16, tag="wv")
    wom_sb = w_pool.tile([P, 2 * D], bf16, tag="wom")

    xk_sb = x_pool.tile([P, DS * S], bf16, tag="xk")
    xq_sb = x_pool.tile([P, DS * S], bf16, tag="xq")
    xv_sb = x_pool.tile([P, DS * S], bf16, tag="xv")

    qT = [qk_pool.tile([P, S], bf16, tag=f"qT{ft}", name=f"qT{ft}") for ft in range(2)]
    kT = [qk_pool.tile([P, S], bf16, tag=f"kT{ft}", name=f"kT{ft}") for ft in range(2)]

    v_sb = v_pool.tile([P, KS * NH * VW], bf16, tag="v")
    v_view = v_sb[:].rearrange("p (ks h w) -> p ks h w", ks=KS, h=NH)
    nc.gpsimd.memset(v_sb[:], 1.0)

    # ---- PE warmup: contentless matmuls to climb the p-state ramp while the
    # first input DMAs are in flight. Results are discarded.
    nc.gpsimd.memset(wu_sb[:], 0.0)
    for wu in range(20):
        ps = pj_pool.tile([P, QW], f32, tag="pj", name=f"wu{wu}")
        nc.tensor.matmul(ps[:, 0:P], wu_sb[:, 0:P], wu_sb[:], start=True, stop=True)

    # ---- DMA prologue (SP queue) ----
    def load_w(w_sb, wname, ng):
        nc.sync.dma_start(
            w_sb[:].rearrange("p (g f) -> p g f", g=ng),
            t[wname].ap().rearrange("(g p) f -> p g f", p=P),
        )

    def load_x_chunk(x_sb, xname, c):
        nc.sync.dma_start(
            x_sb[:].rearrange("p (ds q) -> p ds q", ds=DS)[:, :, c * QW:(c + 1) * QW],
            t[xname].ap().rearrange("(ds p) q -> p ds q", p=P)[:, :, c * QW:(c + 1) * QW],
        )

    nm_tiles = {}

    def load_nm(ph, qq):
        # mask slice for quarter qq: [P, KS, QW], four DMAs of 4 key slices
        # each so early score pairs unblock as soon as their slice lands
        nmt = nm_pool.tile([P, KS * QW], bf16, tag="nm", name=f"nm{ph}_{qq}")
        for quad in range(4):
            nc.sync.dma_start(
                nmt[:].rearrange("p (ks q) -> p ks q", ks=KS)[:, quad * 4:(quad + 1) * 4, :],
                t["nmT"].ap().rearrange("(ks p) q -> p ks q", p=P)[
                    :, quad * 4:(quad + 1) * 4, qq * QW:(qq + 1) * QW],
            )
        nm_tiles[qq] = nmt

    # Ordered by first consumption on the serial DMA device.  The exp stream
    # only needs wk/xk + wq/xq-quarter0; the mask multiply (DVE) and the ctx
    # weave (one session behind) tolerate later arrival, so nm and xv follow
    # the full xk.
    load_w(wk_sb, "wkT", DS)
    load_x_chunk(xk_sb, "xkT", 0)
    load_w(wq_sb, "wqT", DS)
    load_x_chunk(xq_sb, "xqT", 0)
    nc.sync.dma_start(bk_sb[:], t["bk"].ap().rearrange("(ft p) one -> p (ft one)", p=P))
    nc.sync.dma_start(bq_sb[:], t["bq"].ap().rearrange("(ft p) one -> p (ft one)", p=P))
    def load_nm_quad(nmt, qq, quad):
        nc.sync.dma_start(
            nmt[:].rearrange("p (ks q) -> p ks q", ks=KS)[:, quad * 4:(quad + 1) * 4, :],
            t["nmT"].ap().rearrange("(ks p) q -> p ks q", p=P)[
                :, quad * 4:(quad + 1) * 4, qq * QW:(qq + 1) * QW],
        )

    load_x_chunk(xk_sb, "xkT", 1)
    load_x_chunk(xk_sb, "xkT", 2)
    load_x_chunk(xk_sb, "xkT", 3)
    load_w(wv_sb, "wvT", DS)
    nm00 = nm_pool.tile([P, KS * QW], bf16, tag="nm", name="nm0_0")
    nm_tiles[0] = nm00
    load_x_chunk(xv_sb, "xvT", 0)
    load_x_chunk(xq_sb, "xqT", 1)
    load_x_chunk(xv_sb, "xvT", 1)
    load_nm_quad(nm00, 0, 0)
    load_nm_quad(nm00, 0, 1)
    load_x_chunk(xv_sb, "xvT", 2)
    load_nm_quad(nm00, 0, 2)
    load_x_chunk(xv_sb, "xvT", 3)
    load_nm_quad(nm00, 0, 3)
    load_nm(0, 1)
    nc.sync.dma_start(ident_sb[:], t["ident"].ap())
    # nm(0,2)/nm(0,3) park on their buffers' WAR (earlier mask reads), but
    # everything queued behind them here is needed later, so parking on the
    # SP queue is harmless
    load_x_chunk(xq_sb, "xqT", 2)
    load_nm(0, 2)
    load_x_chunk(xq_sb, "xqT", 3)
    load_nm(0, 3)
    load_w(wom_sb, "womT", 2)

    # ---- projection work units, split small so the weave never blocks the
    # score/exp stream for long (PE executes in emission order) ----
    qk_state = {}

    def qk_half(kind, ft, tc4, half):
        w_sb, x_sb = (wk_sb, xk_sb) if kind == "k" else (wq_sb, xq_sb)
        if half == 0:
            ps = pj_pool.tile([P, QW], f32, tag="pj", name=f"pj_{kind}{ft}_{tc4}")
            qk_state[(kind, ft, tc4)] = ps
        else:
            ps = qk_state.pop((kind, ft, tc4))
        for ds in range(half * 4, half * 4 + 4):
            nc.tensor.matmul(
                ps[:],
                w_sb[:, ds * F + ft * P: ds * F + (ft + 1) * P],
                x_sb[:, ds * S + tc4 * QW: ds * S + (tc4 + 1) * QW],
                start=(ds == 0),
                stop=(ds == DS - 1),
            )
        if half == 1:
            out_tiles, b_sb = (kT, bk_sb) if kind == "k" else (qT, bq_sb)
            nc.vector.tensor_scalar_add(
                out_tiles[ft][:, tc4 * QW:(tc4 + 1) * QW], ps[:], b_sb[:, ft:ft + 1]
            )

    def vp_tt(hp, tt):
        # v projection for head pair hp, token slice tt
        ps = pj_pool.tile([P, QW], f32, tag="pj")
        for ds in range(DS):
            nc.tensor.matmul(
                ps[:, 0:P],
                xv_sb[:, ds * S + tt * P: ds * S + (tt + 1) * P],
                wv_sb[:, ds * F + hp * P: ds * F + (hp + 1) * P],
                start=(ds == 0),
                stop=(ds == DS - 1),
            )
        nc.vector.tensor_copy(
            v_view[:, tt, 2 * hp:2 * hp + 2, 0:DH],
            ps[:, 0:P].rearrange("p (h f) -> p h f", h=2),
        )

    # ---- foreign-work weave with deadline forcing ----
    foreign = []
    done = set()

    def _run(key):
        if key[0] == "v":
            vp_tt(key[1], key[2])
        else:
            qk_half(key[0], key[1], key[2], key[3])
        done.add(key)

    def pump(n=1):
        while n > 0 and foreign:
            _run(foreign.pop(0))
            n -= 1

    def require(key):
        while key not in done:
            assert foreign, f"foreign queue empty but {key} missing"
            _run(foreign.pop(0))

    # prologue compute: first key chunk + first query quarter of ft0
    for half in range(2):
        qk_half("k", 0, 0, half)
    for half in range(2):
        qk_half("q", 0, 0, half)
    done.add(("k", 0, 0, 1))
    done.add(("q", 0, 0, 1))

    def _qk_keys(kind, ft, tc4):
        return [(kind, ft, tc4, 0), (kind, ft, tc4, 1)]

    # ordered by DMA readiness and consumption deadline
    foreign.extend(
        _qk_keys("k", 0, 1) + _qk_keys("k", 0, 2) + _qk_keys("k", 0, 3)
        + [("v", 0, tt) for tt in range(KS)]
        + _qk_keys("q", 0, 1)
        + _qk_keys("k", 1, 0) + _qk_keys("k", 1, 1)
        + _qk_keys("k", 1, 2) + _qk_keys("k", 1, 3)
        + _qk_keys("q", 0, 2) + _qk_keys("q", 1, 0) + _qk_keys("q", 1, 1)
        + _qk_keys("q", 0, 3)
        + [("v", 1, tt) for tt in range(KS)]
        + _qk_keys("q", 1, 2) + _qk_keys("q", 1, 3)
    )

    # ---- output-projection work units (woven after each quarter's last norm)
    ctxT = {}       # (qq, fs) -> tile
    cn_tiles = {}   # (h, qq) -> [4 normalized ctx blocks in [q, f] layout]
    opq = []        # pending transpose / output-projection units
    op_pool = None

    def transp_unit(h, qq):
        ft, ro = h // 2, (h % 2) * DH
        if (qq, ft) not in ctxT:
            ctxT[(qq, ft)] = ct_pool.tile([P, QW], bf16, tag=f"ct{ft}", name=f"ct{ft}_{qq}")
        trt = tr_pool.tile([DH, 4 * 2 * P], bf16, tag="tr")
        trv = trt[:].rearrange("p (b w) -> p b w", b=4)
        for qb in range(4):
            nc.tensor.matmul(
                trv[:, qb, 0:P], cn_tiles[(h, qq)][qb][:], ident_sb[:],
                is_transpose=True,
            )
        nc.vector.tensor_copy(
            ctxT[(qq, ft)][ro:ro + DH, :].rearrange("p (b w) -> p b w", b=4),
            trv[:, :, 0:P],
        )

    def op_unit(qq, ot):
        ps = op_pool.tile([P, QW], f32, tag="op")
        for fs in range(2):
            nc.tensor.matmul(
                ps[:],
                wom_sb[:, fs * D + ot * P: fs * D + (ot + 1) * P],
                ctxT[(qq, fs)][:],
                start=(fs == 0),
                stop=(fs == 1),
            )
        dst = t["outT"].ap()[ot * P:(ot + 1) * P, qq * QW:(qq + 1) * QW]
        ob = ob_pool.tile([P, QW], bf16, tag="ob")
        if qq == NQ - 1 and ot % 2 == 1:
            nc.scalar.copy(ob[:], ps[:])   # tail: ACT is idle by then
        else:
            nc.vector.tensor_copy(ob[:], ps[:])
        nc.sync.dma_start(dst, ob[:])

    def run_opq(n):
        while n > 0 and opq:
            kind, h_or_ot, qq = opq.pop(0)
            if kind == "tr":
                transp_unit(h_or_ot, qq)
            else:
                op_unit(qq, h_or_ot)
            n -= 1

    # ---- session machinery: scores/exp/mask for (h, qq); the PREVIOUS
    # session's ctx matmuls and normalization weave into this session's slots
    # so the exp stream never waits on ctx inputs.
    def ctx_qb(st, qb):
        # one q-block's FULL 16-step accumulation as a consecutive run: a
        # PSUM bank supports only one open accumulation group at a time, so
        # groups sharing the cx bank must never interleave
        h = st["h"]
        for tt in range(KS):
            require(("v", h // 2, tt))
        for ks in range(KS):
            pt = st["pts"][ks // 2]
            j = ks % 2
            nc.tensor.matmul(
                st["cxv"][:, qb, 0:VW],
                pt[:, j * QW + qb * P: j * QW + (qb + 1) * P],
                v_view[:, ks, h, :],
                start=(ks == 0),
                stop=(ks == KS - 1),
            )

    tr_open = [False]
    tr_pushed = set()

    def norm(st):
        h, qq, cxv = st["h"], st["qq"], st["cxv"]
        blocks = []
        for qb in range(4):
            rs = rs_pool.tile([P, 1], f32, tag="rs")
            nc.vector.reciprocal(rs[:], cxv[:, qb, DH:DH + 1])
            cn_t = cn_pool.tile([P, DH], bf16, tag="cn", name=f"cn{h}_{qq}_{qb}")
            nc.vector.tensor_scalar_mul(cn_t[:], cxv[:, qb, 0:DH], rs[:])
            if dbg is not None and h == 0:
                nc.sync.dma_start(
                    dbg["cn"].ap()[(qq * 4 + qb) * P:(qq * 4 + qb + 1) * P, :], cn_t[:]
                )
            blocks.append(cn_t)
        cn_tiles[(h, qq)] = blocks
        if tr_open[0]:
            # queue transposes for every normalized head of this quarter
            for hh in range(4):
                if (hh, qq) in cn_tiles and (hh, qq) not in tr_pushed:
                    tr_pushed.add((hh, qq))
                    opq.append(("tr", hh, qq))
        if h == 3:
            for ot in range(8):
                opq.append(("op", ot, qq))

    def run_session(h, qq, prev, quota, ctx_start=0, ctx_end=6, self_ctx=False):
        ft, ro = h // 2, (h % 2) * DH
        require(("q", ft, qq, 1))
        # 4 q-block accumulation regions, each 512B-aligned within one bank
        cx = cx_pool.tile([P, 4 * P], f32, tag="cx")
        st = {
            "h": h, "qq": qq, "pts": [None] * NPAIR,
            "cxv": cx[:].rearrange("p (b w) -> p b w", b=4),
        }
        nmv = nm_tiles[qq][:].rearrange("p (ks q) -> p ks q", ks=KS)
        ctx_slots = [[] for _ in range(NPAIR)]
        if prev is not None:
            nsl = ctx_end - ctx_start
            for qb in range(4):
                ctx_slots[ctx_start + (qb * nsl) // 4].append(qb)
        pumped = 0
        for p in range(NPAIR):
            require(("k", ft, p // 2, 1))
            ps = sc_pool.tile([P, 2 * QW], f32, tag="sc")
            for j in (0, 1):
                ks = 2 * p + j
                nc.tensor.matmul(
                    ps[:, j * QW:(j + 1) * QW],
                    kT[ft][ro:ro + DH, ks * P:(ks + 1) * P],
                    qT[ft][ro:ro + DH, qq * QW:(qq + 1) * QW],
                    start=True,
                    stop=True,
                )
            pt = pt_pool.tile([P, 2 * QW], bf16, tag="pt")
            nc.scalar.activation(pt[:], ps[:], Exp, scale=0.125)
            ptv = pt[:].rearrange("p (j q) -> p j q", j=2)
            nc.vector.tensor_mul(ptv, ptv, nmv[:, 2 * p:2 * p + 2, :])
            st["pts"][p] = pt
            if dbg is not None and h == 0:
                nc.sync.dma_start(
                    dbg["pT0"].ap().rearrange(
                        "(pp two pr) q -> pr pp two q", pr=P, two=2)
                    [:, p, :, qq * QW:(qq + 1) * QW],
                    pt[:].rearrange("p (two q) -> p two q", two=2),
                )
            if prev is not None:
                # keep the v-projection weave paced even before its q-block
                # chunk needs it
                require(("v", prev["h"] // 2, 2 * p))
                require(("v", prev["h"] // 2, 2 * p + 1))
                for qb in ctx_slots[p]:
                    ctx_qb(prev, qb)
                if p == ctx_end:
                    norm(prev)
            if opq:
                run_opq(2)
            elif pumped < quota:
                pump(1)
                pumped += 1
        if prev is not None and ctx_end >= NPAIR:
            norm(prev)
        if self_ctx:
            for qb in range(4):
                ctx_qb(st, qb)
            norm(st)
        return st

    # ---- main schedule: 16 sessions, software-pipelined ----
    order = [(h0 + dh, qq) for ph, h0 in ((1, 0), (2, 2)) for qq in range(NQ)
             for dh in (0, 1)]
    prev = None
    op_cm = tr_cm = None
    for si, (h, qq) in enumerate(order):
        # mask prefetch for phase 2 (phase-1 masks are all in the prologue)
        if (h, qq) == (1, 2):
            load_nm(1, 0)
        elif (h, qq) == (3, 0):
            load_nm(1, 1)
        elif (h, qq) == (3, 1):
            load_nm(1, 2)
        elif (h, qq) == (3, 2):
            load_nm(1, 3)
        if si == 10:
            # all projection work must be done now; swap pj's PSUM banks to
            # the transpose + output-projection pools
            while foreign:
                pump(1)
            pj_cm.__exit__(None, None, None)
            tr_cm = tc.tile_pool(name="tr", bufs=1, space="PSUM")
            tr_pool = tr_cm.__enter__()
            op_cm = tc.tile_pool(name="op", bufs=1, space="PSUM")
            op_pool = op_cm.__enter__()
            tr_open[0] = True
        prev = run_session(h, qq, prev,
                           quota=0 if si <= 1 else (8 if si == 9 else 4),
                           ctx_start=6 if si == 1 else 2,
                           ctx_end=8 if si == 1 else 7,
                           self_ctx=(si == 15))

    # tail: last quarter's transposes, then a double-buffered outproj burst
    while opq and opq[0][0] == "tr":
        kind, hh, qq = opq.pop(0)
        transp_unit(hh, qq)
    op_cm.__exit__(None, None, None)
    tr_cm.__exit__(None, None, None)
    cx_cm.__exit__(None, None, None)
    op_cm = tc.tile_pool(name="op2", bufs=4, space="PSUM")
    op_pool = op_cm.__enter__()
    run_opq(len(opq))

    if dbg is not None:
        for ft in range(2):
            nc.sync.dma_start(dbg["qT"].ap()[ft * P:(ft + 1) * P, :], qT[ft][:])
            nc.sync.dma_start(dbg["kT"].ap()[ft * P:(ft + 1) * P, :], kT[ft][:])
        nc.sync.dma_start(dbg["v"].ap(), v_sb[:])
        for qq in range(NQ):
            for fs in range(2):
                nc.sync.dma_start(
                    dbg["ctxT"].ap()[fs * P:(fs + 1) * P, qq * QW:(qq + 1) * QW],
                    ctxT[(qq, fs)][:],
                )

    assert not foreign and not opq, f"undrained work: {foreign} {opq}"
    op_cm.__exit__(None, None, None)
    es.close()


@functools.lru_cache(maxsize=1)
def _build(debug=False):
    import concourse.bacc as bacc
    import concourse.mybir as mybir
    import concourse.tile as tile

    bf16 = mybir.dt.bfloat16
    f32 = mybir.dt.float32

    nc = bacc.Bacc("TRN2", target_bir_lowering=False, debug=False, num_devices=NCORES)
    t = {
        "xqT": nc.dram_tensor("xqT", (D, S), bf16, kind="ExternalInput"),
        "xkT": nc.dram_tensor("xkT", (D, S), bf16, kind="ExternalInput"),
        "xvT": nc.dram_tensor("xvT", (D, S), bf16, kind="ExternalInput"),
        "wqT": nc.dram_tensor("wqT", (D, F), bf16, kind="ExternalInput"),
        "wkT": nc.dram_tensor("wkT", (D, F), bf16, kind="ExternalInput"),
        "wvT": nc.dram_tensor("wvT", (D, F), bf16, kind="ExternalInput"),
        "womT": nc.dram_tensor("womT", (F, D), bf16, kind="ExternalInput"),
        "nmT": nc.dram_tensor("nmT", (S, S), bf16, kind="ExternalInput"),
        "bq": nc.dram_tensor("bq", (F, 1), f32, kind="ExternalInput"),
        "bk": nc.dram_tensor("bk", (F, 1), f32, kind="ExternalInput"),
        "ident": nc.dram_tensor("ident", (P, P), bf16, kind="ExternalInput"),
        "outT": nc.dram_tensor("outT", (D, S), bf16, kind="ExternalOutput"),
    }
    dbg = None
    if debug:
        dbg = {
            "qT": nc.dram_tensor("dbg_qT", (F, S), bf16, kind="ExternalOutput"),
            "kT": nc.dram_tensor("dbg_kT", (F, S), bf16, kind="ExternalOutput"),
            "v": nc.dram_tensor("dbg_v", (P, KS * NH * VW), bf16, kind="ExternalOutput"),
            "pT0": nc.dram_tensor("dbg_pT0", (S, S), bf16, kind="ExternalOutput"),
            "ctxT": nc.dram_tensor("dbg_ctxT", (F, S), bf16, kind="ExternalOutput"),
            "cn": nc.dram_tensor("dbg_cn", (S, DH), bf16, kind="ExternalOutput"),
        }
    with tile.TileContext(nc) as tc:
        _emit(nc, tc, t, dbg)
    nc.compile()
    return nc


def _prep_core_inputs(c, Q, K, V, mask, Wq, bq, Wk, bk, Wv, Wo, _cache={}):
    import ml_dtypes

    bf = ml_dtypes.bfloat16
    b, g = divmod(c, GROUPS)
    bkey = ("batch", b, id(Q))
    if bkey not in _cache:
        _cache.clear()
        for bb in range(B):
            nm = 1.0 - mask[bb, 0].astype(np.float32)
            _cache[("batch", bb, id(Q))] = {
                "xqT": Q[bb].T.astype(bf),
                "xkT": K[bb].T.astype(bf),
                "xvT": V[bb].T.astype(bf),
                "nmT": nm.T.astype(bf),
            }
    fsl = slice(g * F, (g + 1) * F)
    return {
        **_cache[bkey],
        "wqT": Wq[fsl, :].T.astype(bf),
        "wkT": Wk[fsl, :].T.astype(bf),
        "wvT": Wv[fsl, :].T.astype(bf),
        "womT": Wo[:, fsl].T.astype(bf),
        "bq": bq[fsl].reshape(F, 1).astype(np.float32),
        "bk": bk[fsl].reshape(F, 1).astype(np.float32),
        "ident": np.eye(P, dtype=bf),
    }


def kernel(Q, K, V, mask, Wq, bq, Wk, bk, Wv, bv, Wo, bo, _trace=False, _tmpdir=None):
    from concourse.bass_utils import run_bass_kernel_spmd

    Q, K, V = np.asarray(Q, np.float32), np.asarray(K, np.float32), np.asarray(V, np.float32)
    mask = np.asarray(mask)
    Wq, Wk, Wv, Wo = (np.asarray(w, np.float32) for w in (Wq, Wk, Wv, Wo))
    bq, bk, bv, bo = (np.asarray(x, np.float32) for x in (bq, bk, bv, bo))

    nc = _build()
    in_maps = [_prep_core_inputs(c, Q, K, V, mask, Wq, bq, Wk, bk, Wv, Wo) for c in range(NCORES)]
    kw = {}
    if _trace:
        kw = dict(trace=True, tmpdir=_tmpdir)
    res = run_bass_kernel_spmd(nc, in_maps, core_ids=list(range(NCORES)), **kw)

    const = (Wo @ bv + bo).astype(np.float32)  # softmax rows sum to 1 -> bv enters linearly
    out = np.empty((B, S, D), np.float32)
    for b in range(B):
        acc = res.results[b * GROUPS]["outT"].astype(np.float32)
        for g in range(1, GROUPS):
            acc = acc + res.results[b * GROUPS + g]["outT"].astype(np.float32)
        out[b] = acc.T + const
    if _trace:
        kernel._last_results = res
    return out


# revision 92
# speedup vs baseline: 1.1984x; 1.0064x over previous
"""Multi-head attention (B=2, S=2048, D=1024, H=16) on 8 trn2 NeuronCores.

Sharding: data-parallel over batch (2) x tensor-parallel over head-groups (4).
Core c handles batch c//4, heads [4*(c%4), 4*(c%4)+4).  Each core computes
q/k/v projections for its 256 head-features, masked softmax attention for its
4 heads, and the row-shard of the output projection; partial outputs are
summed on the host during the gather step.

Schedule: attention is blocked into 4 query-quarters of 512.  The ctx matmul
runs with pt as the stationary operand (q on the output partitions, head
features on the free dim), which halves its PE cost versus the v-stationary
orientation; the softmax denominator rides along as a 65th v column and is
divided out with a per-partition reciprocal before a PE transpose puts ctx
back into [feature, q] layout for the output projection.  Projections for
later head groups are woven between attention score pairs so the activation
engine (exp) starts early and stays fed.
"""

import sys
import functools
from contextlib import ExitStack

sys.path.insert(0, "/opt/trn_rl_repo")

import numpy as np

B, S, D, H = 2, 2048, 1024, 16
DH = 64
P = 128
NCORES = 8
GROUPS = 4            # head groups = cores per batch
NH = H // GROUPS      # heads per core = 4
F = NH * DH           # head features per core = 256
KS = S // P           # 16 key slices
DS = D // P           # 8 d_model slices
QW = 512              # query quarter width
NQ = S // QW          # 4 quarters
NPAIR = KS // 2       # 8 key-slice pairs
VW = DH + 1           # v width per head incl. ones column = 65
DEPTH = 4             # ctx lag behind scores, in key-slice pairs


def _emit(nc, tc, t, dbg=None):
    import concourse.mybir as mybir
    bf16 = mybir.dt.bfloat16
    f32 = mybir.dt.float32
    Exp = mybir.ActivationFunctionType.Exp

    es = ExitStack()
    ep = lambda cm: es.enter_context(cm)
    const_pool = ep(tc.tile_pool(name="const", bufs=1))
    w_pool = ep(tc.tile_pool(name="w", bufs=1))
    x_pool = ep(tc.tile_pool(name="x", bufs=1))
    qk_pool = ep(tc.tile_pool(name="qk", bufs=1))
    v_pool = ep(tc.tile_pool(name="v", bufs=1))
    nm_pool = ep(tc.tile_pool(name="nm", bufs=2))
    pt_pool = ep(tc.tile_pool(name="pt", bufs=13))
    cn_pool = ep(tc.tile_pool(name="cn", bufs=40))
    rs_pool = ep(tc.tile_pool(name="rs", bufs=2))
    ct_pool = ep(tc.tile_pool(name="ct", bufs=2))
    ob_pool = ep(tc.tile_pool(name="ob", bufs=4))
    # PSUM banks: sc 2x2 + cx 2x1 + pj 2x1 = 8 during the projection phase;
    # pj's two banks are handed to tr (1) + op (1) once projections finish.
    # pj is opened last so its mid-emit release keeps the pool stack LIFO.
    sc_pool = ep(tc.tile_pool(name="sc", bufs=2, space="PSUM"))
    cx_cm = tc.tile_pool(name="cx", bufs=2, space="PSUM")
    cx_pool = cx_cm.__enter__()
    tr_pool = None
    pj_cm = tc.tile_pool(name="pj", bufs=2, space="PSUM")
    pj_pool = pj_cm.__enter__()

    # ---- SBUF tiles ----
    bq_sb = const_pool.tile([P, 2], f32, tag="bq")
    bk_sb = const_pool.tile([P, 2], f32, tag="bk")
    ident_sb = const_pool.tile([P, P], bf16, tag="ident")
    wu_sb = const_pool.tile([P, 128], bf16, tag="wu")  # warmup garbage

    wq_sb = w_pool.tile([P, DS * F], bf16, tag="wq")
    wk_sb = w_pool.tile([P, DS * F], bf16, tag="wk")
    wv_sb = w_pool.tile([P, DS * F], bf16, tag="wv")
    wom_sb = w_pool.tile([P, 2 * D], bf16, tag="wom")

    xk_sb = x_pool.tile([P, DS * S], bf16, tag="xk")
    xq_sb = x_pool.tile([P, DS * S], bf16, tag="xq")
    xv_sb = x_pool.tile([P, DS * S], bf16, tag="xv")

    qT = [qk_pool.tile([P, S], bf16, tag=f"qT{ft}", name=f"qT{ft}") for ft in range(2)]
    kT = [qk_pool.tile([P, S], bf16, tag=f"kT{ft}", name=f"kT{ft}") for ft in range(2)]

    v_sb = v_pool.tile([P, KS * NH * VW], bf16, tag="v")
    v_view = v_sb[:].rearrange("p (ks h w) -> p ks h w", ks=KS, h=NH)
    nc.gpsimd.memset(v_sb[:], 1.0)

    # ---- PE warmup: contentless matmuls to climb the p-state ramp while the
    # first input DMAs are in flight. Results are discarded.
    nc.gpsimd.memset(wu_sb[:], 0.0)
    for wu in range(20):
        ps = pj_pool.tile([P, QW], f32, tag="pj", name=f"wu{wu}")
        nc.tensor.matmul(ps[:, 0:P], wu_sb[:, 0:P], wu_sb[:], start=True, stop=True)

    # ---- DMA prologue (SP queue) ----
    def load_w(w_sb, wname, ng):
        nc.sync.dma_start(
            w_sb[:].rearrange("p (g f) -> p g f", g=ng),
            t[wname].ap().rearrange("(g p) f -> p g f", p=P),
        )

    def load_x_chunk(x_sb, xname, c):
        nc.sync.dma_start(
            x_sb[:].rearrange("p (ds q) -> p ds q", ds=DS)[:, :, c * QW:(c + 1) * QW],
            t[xname].ap().rearrange("(ds p) q -> p ds q", p=P)[:, :, c * QW:(c + 1) * QW],
        )

    nm_tiles = {}

    def load_nm(ph, qq):
        # mask slice for quarter qq: [P, KS, QW], four DMAs of 4 key slices
        # each so early score pairs unblock as soon as their slice lands
        nmt = nm_pool.tile([P, KS * QW], bf16, tag="nm", name=f"nm{ph}_{qq}")
        for quad in range(4):
            nc.sync.dma_start(
                nmt[:].rearrange("p (ks q) -> p ks q", ks=KS)[:, quad * 4:(quad + 1) * 4, :],
                t["nmT"].ap().rearrange("(ks p) q -> p ks q", p=P)[
                    :, quad * 4:(quad + 1) * 4, qq * QW:(qq + 1) * QW],
            )
        nm_tiles[qq] = nmt

    # Ordered by first consumption on the serial DMA device.  The exp stream
    # only needs wk/xk + wq/xq-quarter0; the mask multiply (DVE) and the ctx
    # weave (one session behind) tolerate later arrival, so nm and xv follow
    # the full xk.
    load_w(wk_sb, "wkT", DS)
    load_x_chunk(xk_sb, "xkT", 0)
    load_w(wq_sb, "wqT", DS)
    load_x_chunk(xq_sb, "xqT", 0)
    nc.sync.dma_start(bk_sb[:], t["bk"].ap().rearrange("(ft p) one -> p (ft one)", p=P))
    nc.sync.dma_start(bq_sb[:], t["bq"].ap().rearrange("(ft p) one -> p (ft one)", p=P))
    def load_nm_quad(nmt, qq, quad):
        nc.sync.dma_start(
            nmt[:].rearrange("p (ks q) -> p ks q", ks=KS)[:, quad * 4:(quad + 1) * 4, :],
            t["nmT"].ap().rearrange("(ks p) q -> p ks q", p=P)[
                :, quad * 4:(quad + 1) * 4, qq * QW:(qq + 1) * QW],
        )

    load_x_chunk(xk_sb, "xkT", 1)
    load_x_chunk(xk_sb, "xkT", 2)
    load_x_chunk(xk_sb, "xkT", 3)
    load_w(wv_sb, "wvT", DS)
    nm00 = nm_pool.tile([P, KS * QW], bf16, tag="nm", name="nm0_0")
    nm_tiles[0] = nm00
    load_x_chunk(xv_sb, "xvT", 0)
    load_x_chunk(xq_sb, "xqT", 1)
    load_x_chunk(xv_sb, "xvT", 1)
    load_nm_quad(nm00, 0, 0)
    load_nm_quad(nm00, 0, 1)
    load_x_chunk(xv_sb, "xvT", 2)
    load_nm_quad(nm00, 0, 2)
    load_x_chunk(xv_sb, "xvT", 3)
    load_nm_quad(nm00, 0, 3)
    load_nm(0, 1)
    nc.sync.dma_start(ident_sb[:], t["ident"].ap())
    # nm(0,2)/nm(0,3) park on their buffers' WAR (earlier mask reads), but
    # everything queued behind them here is needed later, so parking on the
    # SP queue is harmless
    load_x_chunk(xq_sb, "xqT", 2)
    load_nm(0, 2)
    load_x_chunk(xq_sb, "xqT", 3)
    load_nm(0, 3)
    load_w(wom_sb, "womT", 2)

    # ---- projection work units, split small so the weave never blocks the
    # score/exp stream for long (PE executes in emission order) ----
    qk_state = {}

    def qk_half(kind, ft, tc4, half):
        w_sb, x_sb = (wk_sb, xk_sb) if kind == "k" else (wq_sb, xq_sb)
        if half == 0:
            ps = pj_pool.tile([P, QW], f32, tag="pj", name=f"pj_{kind}{ft}_{tc4}")
            qk_state[(kind, ft, tc4)] = ps
        else:
            ps = qk_state.pop((kind, ft, tc4))
        for ds in range(half * 4, half * 4 + 4):
            nc.tensor.matmul(
                ps[:],
                w_sb[:, ds * F + ft * P: ds * F + (ft + 1) * P],
                x_sb[:, ds * S + tc4 * QW: ds * S + (tc4 + 1) * QW],
                start=(ds == 0),
                stop=(ds == DS - 1),
            )
        if half == 1:
            out_tiles, b_sb = (kT, bk_sb) if kind == "k" else (qT, bq_sb)
            nc.vector.tensor_scalar_add(
                out_tiles[ft][:, tc4 * QW:(tc4 + 1) * QW], ps[:], b_sb[:, ft:ft + 1]
            )

    def vp_tt(hp, tt):
        # v projection for head pair hp, token slice tt
        ps = pj_pool.tile([P, QW], f32, tag="pj")
        for ds in range(DS):
            nc.tensor.matmul(
                ps[:, 0:P],
                xv_sb[:, ds * S + tt * P: ds * S + (tt + 1) * P],
                wv_sb[:, ds * F + hp * P: ds * F + (hp + 1) * P],
                start=(ds == 0),
                stop=(ds == DS - 1),
            )
        nc.vector.tensor_copy(
            v_view[:, tt, 2 * hp:2 * hp + 2, 0:DH],
            ps[:, 0:P].rearrange("p (h f) -> p h f", h=2),
        )

    # ---- foreign-work weave with deadline forcing ----
    foreign = []
    done = set()

    def _run(key):
        if key[0] == "v":
            vp_tt(key[1], key[2])
        else:
            qk_half(key[0], key[1], key[2], key[3])
        done.add(key)

    def pump(n=1):
        while n > 0 and foreign:
            _run(foreign.pop(0))
            n -= 1

    def require(key):
        while key not in done:
            assert foreign, f"foreign queue empty but {key} missing"
            _run(foreign.pop(0))

    # prologue compute: first key chunk + first query quarter of ft0
    for half in range(2):
        qk_half("k", 0, 0, half)
    for half in range(2):
        qk_half("q", 0, 0, half)
    done.add(("k", 0, 0, 1))
    done.add(("q", 0, 0, 1))

    def _qk_keys(kind, ft, tc4):
        return [(kind, ft, tc4, 0), (kind, ft, tc4, 1)]

    # ordered by DMA readiness and consumption deadline
    foreign.extend(
        _qk_keys("k", 0, 1) + _qk_keys("k", 0, 2) + _qk_keys("k", 0, 3)
        + [("v", 0, tt) for tt in range(KS)]
        + _qk_keys("q", 0, 1)
        + _qk_keys("k", 1, 0) + _qk_keys("k", 1, 1)
        + _qk_keys("k", 1, 2) + _qk_keys("k", 1, 3)
        + _qk_keys("q", 0, 2) + _qk_keys("q", 1, 0) + _qk_keys("q", 1, 1)
        + _qk_keys("q", 0, 3)
        + [("v", 1, tt) for tt in range(KS)]
        + _qk_keys("q", 1, 2) + _qk_keys("q", 1, 3)
    )

    # ---- output-projection work units (woven after each quarter's last norm)
    ctxT = {}       # (qq, fs) -> tile
    cn_tiles = {}   # (h, qq) -> [4 normalized ctx blocks in [q, f] layout]
    opq = []        # pending transpose / output-projection units
    op_pool = None

    def transp_unit(h, qq):
        ft, ro = h // 2, (h % 2) * DH
        if (qq, ft) not in ctxT:
            ctxT[(qq, ft)] = ct_pool.tile([P, QW], bf16, tag=f"ct{ft}", name=f"ct{ft}_{qq}")
        trt = tr_pool.tile([DH, 4 * 2 * P], bf16, tag="tr")
        trv = trt[:].rearrange("p (b w) -> p b w", b=4)
        for qb in range(4):
            nc.tensor.matmul(
                trv[:, qb, 0:P], cn_tiles[(h, qq)][qb][:], ident_sb[:],
                is_transpose=True,
            )
        nc.vector.tensor_copy(
            ctxT[(qq, ft)][ro:ro + DH, :].rearrange("p (b w) -> p b w", b=4),
            trv[:, :, 0:P],
        )

    def op_unit(qq, ot):
        ps = op_pool.tile([P, QW], f32, tag="op")
        for fs in range(2):
            nc.tensor.matmul(
                ps[:],
                wom_sb[:, fs * D + ot * P: fs * D + (ot + 1) * P],
                ctxT[(qq, fs)][:],
                start=(fs == 0),
                stop=(fs == 1),
            )
        dst = t["outT"].ap()[ot * P:(ot + 1) * P, qq * QW:(qq + 1) * QW]
        ob = ob_pool.tile([P, QW], bf16, tag="ob")
        if qq == NQ - 1 and ot % 2 == 1:
            nc.scalar.copy(ob[:], ps[:])   # tail: ACT is idle by then
        else:
            nc.vector.tensor_copy(ob[:], ps[:])
        nc.sync.dma_start(dst, ob[:])

    def run_opq(n):
        while n > 0 and opq:
            kind, h_or_ot, qq = opq.pop(0)
            if kind == "tr":
                transp_unit(h_or_ot, qq)
            else:
                op_unit(qq, h_or_ot)
            n -= 1

    # ---- session machinery: scores/exp/mask for (h, qq); the PREVIOUS
    # session's ctx matmuls and normalization weave into this session's slots
    # so the exp stream never waits on ctx inputs.
    def ctx_qb(st, qb):
        # one q-block's FULL 16-step accumulation as a consecutive run: a
        # PSUM bank supports only one open accumulation group at a time, so
        # groups sharing the cx bank must never interleave
        h = st["h"]
        for tt in range(KS):
            require(("v", h // 2, tt))
        for ks in range(KS):
            pt = st["pts"][ks // 2]
            j = ks % 2
            nc.tensor.matmul(
                st["cxv"][:, qb, 0:VW],
                pt[:, j * QW + qb * P: j * QW + (qb + 1) * P],
                v_view[:, ks, h, :],
                start=(ks == 0),
                stop=(ks == KS - 1),
            )

    tr_open = [False]
    tr_pushed = set()

    def norm(st):
        h, qq, cxv = st["h"], st["qq"], st["cxv"]
        blocks = []
        for qb in range(4):
            rs = rs_pool.tile([P, 1], f32, tag="rs")
            nc.vector.reciprocal(rs[:], cxv[:, qb, DH:DH + 1])
            cn_t = cn_pool.tile([P, DH], bf16, tag="cn", name=f"cn{h}_{qq}_{qb}")
            nc.vector.tensor_scalar_mul(cn_t[:], cxv[:, qb, 0:DH], rs[:])
            if dbg is not None and h == 0:
                nc.sync.dma_start(
                    dbg["cn"].ap()[(qq * 4 + qb) * P:(qq * 4 + qb + 1) * P, :], cn_t[:]
                )
            blocks.append(cn_t)
        cn_tiles[(h, qq)] = blocks
        if tr_open[0]:
            # queue transposes for every normalized head of this quarter
            for hh in range(4):
                if (hh, qq) in cn_tiles and (hh, qq) not in tr_pushed:
                    tr_pushed.add((hh, qq))
                    opq.append(("tr", hh, qq))
        if h == 3:
            for ot in range(8):
                opq.append(("op", ot, qq))

    def run_session(h, qq, prev, quota, ctx_start=0, ctx_end=6, self_ctx=False):
        ft, ro = h // 2, (h % 2) * DH
        require(("q", ft, qq, 1))
        # 4 q-block accumulation regions, each 512B-aligned within one bank
        cx = cx_pool.tile([P, 4 * P], f32, tag="cx")
        st = {
            "h": h, "qq": qq, "pts": [None] * NPAIR,
            "cxv": cx[:].rearrange("p (b w) -> p b w", b=4),
        }
        nmv = nm_tiles[qq][:].rearrange("p (ks q) -> p ks q", ks=KS)
        ctx_slots = [[] for _ in range(NPAIR)]
        if prev is not None:
            nsl = ctx_end - ctx_start
            for qb in range(4):
                ctx_slots[ctx_start + (qb * nsl) // 4].append(qb)
        pumped = 0
        for p in range(NPAIR):
            require(("k", ft, p // 2, 1))
            ps = sc_pool.tile([P, 2 * QW], f32, tag="sc")
            for j in (0, 1):
                ks = 2 * p + j
                nc.tensor.matmul(
                    ps[:, j * QW:(j + 1) * QW],
                    kT[ft][ro:ro + DH, ks * P:(ks + 1) * P],
                    qT[ft][ro:ro + DH, qq * QW:(qq + 1) * QW],
                    start=True,
                    stop=True,
                )
            pt = pt_pool.tile([P, 2 * QW], bf16, tag="pt")
            nc.scalar.activation(pt[:], ps[:], Exp, scale=0.125)
            ptv = pt[:].rearrange("p (j q) -> p j q", j=2)
            nc.vector.tensor_mul(ptv, ptv, nmv[:, 2 * p:2 * p + 2, :])
            st["pts"][p] = pt
            if dbg is not None and h == 0:
                nc.sync.dma_start(
                    dbg["pT0"].ap().rearrange(
                        "(pp two pr) q -> pr pp two q", pr=P, two=2)
                    [:, p, :, qq * QW:(qq + 1) * QW],
                    pt[:].rearrange("p (two q) -> p two q", two=2),
                )
            if prev is not None:
                # keep the v-projection weave paced even before its q-block
                # chunk needs it
                require(("v", prev["h"] // 2, 2 * p))
                require(("v", prev["h"] // 2, 2 * p + 1))
                for qb in ctx_slots[p]:
                    ctx_qb(prev, qb)
                if p == ctx_end:
                    norm(prev)
            if opq:
                run_opq(2)
            elif pumped < quota:
                pump(1)
                pumped += 1
        if prev is not None and ctx_end >= NPAIR:
            norm(prev)
        if self_ctx:
            for qb in range(4):
                ctx_qb(st, qb)
            norm(st)
        return st

    # ---- main schedule: 16 sessions, software-pipelined ----
    order = [(h0 + dh, qq) for ph, h0 in ((1, 0), (2, 2)) for qq in range(NQ)
             for dh in (0, 1)]
    prev = None
    op_cm = tr_cm = None
    for si, (h, qq) in enumerate(order):
        # mask prefetch for phase 2 (phase-1 masks are all in the prologue)
        if (h, qq) == (1, 2):
            load_nm(1, 0)
        elif (h, qq) == (3, 0):
            load_nm(1, 1)
        elif (h, qq) == (3, 1):
            load_nm(1, 2)
        elif (h, qq) == (3, 2):
            load_nm(1, 3)
        if si == 10:
            # all projection work must be done now; swap pj's PSUM banks to
            # the transpose + output-projection pools
            while foreign:
                pump(1)
            pj_cm.__exit__(None, None, None)
            tr_cm = tc.tile_pool(name="tr", bufs=1, space="PSUM")
            tr_pool = tr_cm.__enter__()
            op_cm = tc.tile_pool(name="op", bufs=1, space="PSUM")
            op_pool = op_cm.__enter__()
            tr_open[0] = True
        prev = run_session(h, qq, prev,
                           quota=0 if si <= 1 else (8 if si == 9 else 2),
                           ctx_start=6 if si == 1 else 2,
                           ctx_end=8 if si == 1 else 7,
                           self_ctx=(si == 15))

    # tail: last quarter's transposes, then a double-buffered outproj burst
    while opq and opq[0][0] == "tr":
        kind, hh, qq = opq.pop(0)
        transp_unit(hh, qq)
    op_cm.__exit__(None, None, None)
    tr_cm.__exit__(None, None, None)
    cx_cm.__exit__(None, None, None)
    op_cm = tc.tile_pool(name="op2", bufs=4, space="PSUM")
    op_pool = op_cm.__enter__()
    run_opq(len(opq))

    if dbg is not None:
        for ft in range(2):
            nc.sync.dma_start(dbg["qT"].ap()[ft * P:(ft + 1) * P, :], qT[ft][:])
            nc.sync.dma_start(dbg["kT"].ap()[ft * P:(ft + 1) * P, :], kT[ft][:])
        nc.sync.dma_start(dbg["v"].ap(), v_sb[:])
        for qq in range(NQ):
            for fs in range(2):
                nc.sync.dma_start(
                    dbg["ctxT"].ap()[fs * P:(fs + 1) * P, qq * QW:(qq + 1) * QW],
                    ctxT[(qq, fs)][:],
                )

    assert not foreign and not opq, f"undrained work: {foreign} {opq}"
    op_cm.__exit__(None, None, None)
    es.close()


@functools.lru_cache(maxsize=1)
def _build(debug=False):
    import concourse.bacc as bacc
    import concourse.mybir as mybir
    import concourse.tile as tile

    bf16 = mybir.dt.bfloat16
    f32 = mybir.dt.float32

    nc = bacc.Bacc("TRN2", target_bir_lowering=False, debug=False, num_devices=NCORES)
    t = {
        "xqT": nc.dram_tensor("xqT", (D, S), bf16, kind="ExternalInput"),
        "xkT": nc.dram_tensor("xkT", (D, S), bf16, kind="ExternalInput"),
        "xvT": nc.dram_tensor("xvT", (D, S), bf16, kind="ExternalInput"),
        "wqT": nc.dram_tensor("wqT", (D, F), bf16, kind="ExternalInput"),
        "wkT": nc.dram_tensor("wkT", (D, F), bf16, kind="ExternalInput"),
        "wvT": nc.dram_tensor("wvT", (D, F), bf16, kind="ExternalInput"),
        "womT": nc.dram_tensor("womT", (F, D), bf16, kind="ExternalInput"),
        "nmT": nc.dram_tensor("nmT", (S, S), bf16, kind="ExternalInput"),
        "bq": nc.dram_tensor("bq", (F, 1), f32, kind="ExternalInput"),
        "bk": nc.dram_tensor("bk", (F, 1), f32, kind="ExternalInput"),
        "ident": nc.dram_tensor("ident", (P, P), bf16, kind="ExternalInput"),
        "outT": nc.dram_tensor("outT", (D, S), bf16, kind="ExternalOutput"),
    }
    dbg = None
    if debug:
        dbg = {
            "qT": nc.dram_tensor("dbg_qT", (F, S), bf16, kind="ExternalOutput"),
            "kT": nc.dram_tensor("dbg_kT", (F, S), bf16, kind="ExternalOutput"),
            "v": nc.dram_tensor("dbg_v", (P, KS * NH * VW), bf16, kind="ExternalOutput"),
            "pT0": nc.dram_tensor("dbg_pT0", (S, S), bf16, kind="ExternalOutput"),
            "ctxT": nc.dram_tensor("dbg_ctxT", (F, S), bf16, kind="ExternalOutput"),
            "cn": nc.dram_tensor("dbg_cn", (S, DH), bf16, kind="ExternalOutput"),
        }
    with tile.TileContext(nc) as tc:
        _emit(nc, tc, t, dbg)
    nc.compile()
    return nc


def _prep_core_inputs(c, Q, K, V, mask, Wq, bq, Wk, bk, Wv, Wo, _cache={}):
    import ml_dtypes

    bf = ml_dtypes.bfloat16
    b, g = divmod(c, GROUPS)
    bkey = ("batch", b, id(Q))
    if bkey not in _cache:
        _cache.clear()
        for bb in range(B):
            nm = 1.0 - mask[bb, 0].astype(np.float32)
            _cache[("batch", bb, id(Q))] = {
                "xqT": Q[bb].T.astype(bf),
                "xkT": K[bb].T.astype(bf),
                "xvT": V[bb].T.astype(bf),
                "nmT": nm.T.astype(bf),
            }
    fsl = slice(g * F, (g + 1) * F)
    return {
        **_cache[bkey],
        "wqT": Wq[fsl, :].T.astype(bf),
        "wkT": Wk[fsl, :].T.astype(bf),
        "wvT": Wv[fsl, :].T.astype(bf),
        "womT": Wo[:, fsl].T.astype(bf),
        "bq": bq[fsl].reshape(F, 1).astype(np.float32),
        "bk": bk[fsl].reshape(F, 1).astype(np.float32),
        "ident": np.eye(P, dtype=bf),
    }


def kernel(Q, K, V, mask, Wq, bq, Wk, bk, Wv, bv, Wo, bo, _trace=False, _tmpdir=None):
    from concourse.bass_utils import run_bass_kernel_spmd

    Q, K, V = np.asarray(Q, np.float32), np.asarray(K, np.float32), np.asarray(V, np.float32)
    mask = np.asarray(mask)
    Wq, Wk, Wv, Wo = (np.asarray(w, np.float32) for w in (Wq, Wk, Wv, Wo))
    bq, bk, bv, bo = (np.asarray(x, np.float32) for x in (bq, bk, bv, bo))

    nc = _build()
    in_maps = [_prep_core_inputs(c, Q, K, V, mask, Wq, bq, Wk, bk, Wv, Wo) for c in range(NCORES)]
    kw = {}
    if _trace:
        kw = dict(trace=True, tmpdir=_tmpdir)
    res = run_bass_kernel_spmd(nc, in_maps, core_ids=list(range(NCORES)), **kw)

    const = (Wo @ bv + bo).astype(np.float32)  # softmax rows sum to 1 -> bv enters linearly
    out = np.empty((B, S, D), np.float32)
    for b in range(B):
        acc = res.results[b * GROUPS]["outT"].astype(np.float32)
        for g in range(1, GROUPS):
            acc = acc + res.results[b * GROUPS + g]["outT"].astype(np.float32)
        out[b] = acc.T + const
    if _trace:
        kernel._last_results = res
    return out


# revision 97
# speedup vs baseline: 1.2005x; 1.0018x over previous
"""Multi-head attention (B=2, S=2048, D=1024, H=16) on 8 trn2 NeuronCores.

Sharding: data-parallel over batch (2) x tensor-parallel over head-groups (4).
Core c handles batch c//4, heads [4*(c%4), 4*(c%4)+4).  Each core computes
q/k/v projections for its 256 head-features, masked softmax attention for its
4 heads, and the row-shard of the output projection; partial outputs are
summed on the host during the gather step.

Schedule: attention is blocked into 4 query-quarters of 512.  The ctx matmul
runs with pt as the stationary operand (q on the output partitions, head
features on the free dim), which halves its PE cost versus the v-stationary
orientation; the softmax denominator rides along as a 65th v column and is
divided out with a per-partition reciprocal before a PE transpose puts ctx
back into [feature, q] layout for the output projection.  Projections for
later head groups are woven between attention score pairs so the activation
engine (exp) starts early and stays fed.
"""

import sys
import functools
from contextlib import ExitStack

sys.path.insert(0, "/opt/trn_rl_repo")

import numpy as np

B, S, D, H = 2, 2048, 1024, 16
DH = 64
P = 128
NCORES = 8
GROUPS = 4            # head groups = cores per batch
NH = H // GROUPS      # heads per core = 4
F = NH * DH           # head features per core = 256
KS = S // P           # 16 key slices
DS = D // P           # 8 d_model slices
QW = 512              # query quarter width
NQ = S // QW          # 4 quarters
NPAIR = KS // 2       # 8 key-slice pairs
VW = DH + 1           # v width per head incl. ones column = 65
DEPTH = 4             # ctx lag behind scores, in key-slice pairs


def _emit(nc, tc, t, dbg=None):
    import concourse.mybir as mybir
    bf16 = mybir.dt.bfloat16
    f32 = mybir.dt.float32
    Exp = mybir.ActivationFunctionType.Exp

    es = ExitStack()
    ep = lambda cm: es.enter_context(cm)
    const_pool = ep(tc.tile_pool(name="const", bufs=1))
    w_pool = ep(tc.tile_pool(name="w", bufs=1))
    x_pool = ep(tc.tile_pool(name="x", bufs=1))
    qk_pool = ep(tc.tile_pool(name="qk", bufs=1))
    v_pool = ep(tc.tile_pool(name="v", bufs=1))
    nm_pool = ep(tc.tile_pool(name="nm", bufs=2))
    pt_pool = ep(tc.tile_pool(name="pt", bufs=13))
    cn_pool = ep(tc.tile_pool(name="cn", bufs=40))
    rs_pool = ep(tc.tile_pool(name="rs", bufs=2))
    ct_pool = ep(tc.tile_pool(name="ct", bufs=2))
    ob_pool = ep(tc.tile_pool(name="ob", bufs=4))
    # PSUM banks: sc 2x2 + cx 2x1 + pj 2x1 = 8 during the projection phase;
    # pj's two banks are handed to tr (1) + op (1) once projections finish.
    # pj is opened last so its mid-emit release keeps the pool stack LIFO.
    sc_pool = ep(tc.tile_pool(name="sc", bufs=2, space="PSUM"))
    cx_cm = tc.tile_pool(name="cx", bufs=2, space="PSUM")
    cx_pool = cx_cm.__enter__()
    tr_pool = None
    pj_cm = tc.tile_pool(name="pj", bufs=2, space="PSUM")
    pj_pool = pj_cm.__enter__()

    # ---- SBUF tiles ----
    bq_sb = const_pool.tile([P, 2], f32, tag="bq")
    bk_sb = const_pool.tile([P, 2], f32, tag="bk")
    ident_sb = const_pool.tile([P, P], bf16, tag="ident")
    wu_sb = const_pool.tile([P, 128], bf16, tag="wu")  # warmup garbage

    wq_sb = w_pool.tile([P, DS * F], bf16, tag="wq")
    wk_sb = w_pool.tile([P, DS * F], bf16, tag="wk")
    wv_sb = w_pool.tile([P, DS * F], bf16, tag="wv")
    wom_sb = w_pool.tile([P, 2 * D], bf16, tag="wom")

    xk_sb = x_pool.tile([P, DS * S], bf16, tag="xk")
    xq_sb = x_pool.tile([P, DS * S], bf16, tag="xq")
    xv_sb = x_pool.tile([P, DS * S], bf16, tag="xv")

    qT = [qk_pool.tile([P, S], bf16, tag=f"qT{ft}", name=f"qT{ft}") for ft in range(2)]
    kT = [qk_pool.tile([P, S], bf16, tag=f"kT{ft}", name=f"kT{ft}") for ft in range(2)]

    v_sb = v_pool.tile([P, KS * NH * VW], bf16, tag="v")
    v_view = v_sb[:].rearrange("p (ks h w) -> p ks h w", ks=KS, h=NH)
    nc.gpsimd.memset(v_sb[:], 1.0)

    # ---- PE warmup: contentless matmuls to climb the p-state ramp while the
    # first input DMAs are in flight. Results are discarded.
    nc.gpsimd.memset(wu_sb[:], 0.0)
    for wu in range(20):
        ps = pj_pool.tile([P, QW], f32, tag="pj", name=f"wu{wu}")
        nc.tensor.matmul(ps[:, 0:P], wu_sb[:, 0:P], wu_sb[:], start=True, stop=True)

    # ---- DMA prologue (SP queue) ----
    def load_w(w_sb, wname, ng):
        nc.sync.dma_start(
            w_sb[:].rearrange("p (g f) -> p g f", g=ng),
            t[wname].ap().rearrange("(g p) f -> p g f", p=P),
        )

    def load_x_chunk(x_sb, xname, c):
        nc.sync.dma_start(
            x_sb[:].rearrange("p (ds q) -> p ds q", ds=DS)[:, :, c * QW:(c + 1) * QW],
            t[xname].ap().rearrange("(ds p) q -> p ds q", p=P)[:, :, c * QW:(c + 1) * QW],
        )

    nm_tiles = {}

    def load_nm(ph, qq):
        # mask slice for quarter qq: [P, KS, QW], four DMAs of 4 key slices
        # each so early score pairs unblock as soon as their slice lands
        nmt = nm_pool.tile([P, KS * QW], bf16, tag="nm", name=f"nm{ph}_{qq}")
        for quad in range(4):
            nc.sync.dma_start(
                nmt[:].rearrange("p (ks q) -> p ks q", ks=KS)[:, quad * 4:(quad + 1) * 4, :],
                t["nmT"].ap().rearrange("(ks p) q -> p ks q", p=P)[
                    :, quad * 4:(quad + 1) * 4, qq * QW:(qq + 1) * QW],
            )
        nm_tiles[qq] = nmt

    # Ordered by first consumption on the serial DMA device.  The exp stream
    # only needs wk/xk + wq/xq-quarter0; the mask multiply (DVE) and the ctx
    # weave (one session behind) tolerate later arrival, so nm and xv follow
    # the full xk.
    load_w(wk_sb, "wkT", DS)
    load_x_chunk(xk_sb, "xkT", 0)
    load_w(wq_sb, "wqT", DS)
    load_x_chunk(xq_sb, "xqT", 0)
    nc.sync.dma_start(bk_sb[:], t["bk"].ap().rearrange("(ft p) one -> p (ft one)", p=P))
    nc.sync.dma_start(bq_sb[:], t["bq"].ap().rearrange("(ft p) one -> p (ft one)", p=P))
    def load_nm_quad(nmt, qq, quad):
        nc.sync.dma_start(
            nmt[:].rearrange("p (ks q) -> p ks q", ks=KS)[:, quad * 4:(quad + 1) * 4, :],
            t["nmT"].ap().rearrange("(ks p) q -> p ks q", p=P)[
                :, quad * 4:(quad + 1) * 4, qq * QW:(qq + 1) * QW],
        )

    load_x_chunk(xk_sb, "xkT", 1)
    load_x_chunk(xk_sb, "xkT", 2)
    load_x_chunk(xk_sb, "xkT", 3)
    load_w(wv_sb, "wvT", DS)
    nm00 = nm_pool.tile([P, KS * QW], bf16, tag="nm", name="nm0_0")
    nm_tiles[0] = nm00
    load_x_chunk(xv_sb, "xvT", 0)
    load_x_chunk(xq_sb, "xqT", 1)
    load_x_chunk(xv_sb, "xvT", 1)
    load_nm_quad(nm00, 0, 0)
    load_nm_quad(nm00, 0, 1)
    load_x_chunk(xv_sb, "xvT", 2)
    load_nm_quad(nm00, 0, 2)
    load_x_chunk(xv_sb, "xvT", 3)
    load_nm_quad(nm00, 0, 3)
    load_nm(0, 1)
    nc.sync.dma_start(ident_sb[:], t["ident"].ap())
    # nm(0,2)/nm(0,3) park on their buffers' WAR (earlier mask reads), but
    # everything queued behind them here is needed later, so parking on the
    # SP queue is harmless
    load_x_chunk(xq_sb, "xqT", 2)
    load_nm(0, 2)
    load_x_chunk(xq_sb, "xqT", 3)
    load_nm(0, 3)
    load_w(wom_sb, "womT", 2)

    # ---- projection work units, split small so the weave never blocks the
    # score/exp stream for long (PE executes in emission order) ----
    qk_state = {}

    def qk_half(kind, ft, tc4, half):
        w_sb, x_sb = (wk_sb, xk_sb) if kind == "k" else (wq_sb, xq_sb)
        if half == 0:
            ps = pj_pool.tile([P, QW], f32, tag="pj", name=f"pj_{kind}{ft}_{tc4}")
            qk_state[(kind, ft, tc4)] = ps
        else:
            ps = qk_state.pop((kind, ft, tc4))
        for ds in range(half * 4, half * 4 + 4):
            nc.tensor.matmul(
                ps[:],
                w_sb[:, ds * F + ft * P: ds * F + (ft + 1) * P],
                x_sb[:, ds * S + tc4 * QW: ds * S + (tc4 + 1) * QW],
                start=(ds == 0),
                stop=(ds == DS - 1),
            )
        if half == 1:
            out_tiles, b_sb = (kT, bk_sb) if kind == "k" else (qT, bq_sb)
            nc.vector.tensor_scalar_add(
                out_tiles[ft][:, tc4 * QW:(tc4 + 1) * QW], ps[:], b_sb[:, ft:ft + 1]
            )

    def vp_tt(hp, tt):
        # v projection for head pair hp, token slice tt
        ps = pj_pool.tile([P, QW], f32, tag="pj")
        for ds in range(DS):
            nc.tensor.matmul(
                ps[:, 0:P],
                xv_sb[:, ds * S + tt * P: ds * S + (tt + 1) * P],
                wv_sb[:, ds * F + hp * P: ds * F + (hp + 1) * P],
                start=(ds == 0),
                stop=(ds == DS - 1),
            )
        nc.vector.tensor_copy(
            v_view[:, tt, 2 * hp:2 * hp + 2, 0:DH],
            ps[:, 0:P].rearrange("p (h f) -> p h f", h=2),
        )

    # ---- foreign-work weave with deadline forcing ----
    foreign = []
    done = set()

    def _run(key):
        if key[0] == "v":
            vp_tt(key[1], key[2])
        else:
            qk_half(key[0], key[1], key[2], key[3])
        done.add(key)

    def pump(n=1):
        while n > 0 and foreign:
            _run(foreign.pop(0))
            n -= 1

    def require(key):
        while key not in done:
            assert foreign, f"foreign queue empty but {key} missing"
            _run(foreign.pop(0))

    # prologue compute: first key chunk + first query quarter of ft0
    for half in range(2):
        qk_half("k", 0, 0, half)
    for half in range(2):
        qk_half("q", 0, 0, half)
    done.add(("k", 0, 0, 1))
    done.add(("q", 0, 0, 1))

    def _qk_keys(kind, ft, tc4):
        return [(kind, ft, tc4, 0), (kind, ft, tc4, 1)]

    # ordered by DMA readiness and consumption deadline
    foreign.extend(
        _qk_keys("k", 0, 1) + _qk_keys("k", 0, 2) + _qk_keys("k", 0, 3)
        + [("v", 0, tt) for tt in range(KS)]
        + _qk_keys("q", 0, 1)
        + _qk_keys("k", 1, 0) + _qk_keys("k", 1, 1)
        + _qk_keys("k", 1, 2) + _qk_keys("k", 1, 3)
        + _qk_keys("q", 0, 2) + _qk_keys("q", 1, 0) + _qk_keys("q", 1, 1)
        + _qk_keys("q", 0, 3)
        + [("v", 1, tt) for tt in range(KS)]
        + _qk_keys("q", 1, 2) + _qk_keys("q", 1, 3)
    )

    # ---- output-projection work units (woven after each quarter's last norm)
    ctxT = {}       # (qq, fs) -> tile
    cn_tiles = {}   # (h, qq) -> [4 normalized ctx blocks in [q, f] layout]
    opq = []        # pending transpose / output-projection units
    op_pool = None

    def transp_unit(h, qq):
        ft, ro = h // 2, (h % 2) * DH
        if (qq, ft) not in ctxT:
            ctxT[(qq, ft)] = ct_pool.tile([P, QW], bf16, tag=f"ct{ft}", name=f"ct{ft}_{qq}")
        trt = tr_pool.tile([DH, 4 * 2 * P], bf16, tag="tr")
        trv = trt[:].rearrange("p (b w) -> p b w", b=4)
        for qb in range(4):
            nc.tensor.matmul(
                trv[:, qb, 0:P], cn_tiles[(h, qq)][qb][:], ident_sb[:],
                is_transpose=True,
            )
        nc.vector.tensor_copy(
            ctxT[(qq, ft)][ro:ro + DH, :].rearrange("p (b w) -> p b w", b=4),
            trv[:, :, 0:P],
        )

    def op_unit(qq, ot):
        ps = op_pool.tile([P, QW], f32, tag="op")
        for fs in range(2):
            nc.tensor.matmul(
                ps[:],
                wom_sb[:, fs * D + ot * P: fs * D + (ot + 1) * P],
                ctxT[(qq, fs)][:],
                start=(fs == 0),
                stop=(fs == 1),
            )
        dst = t["outT"].ap()[ot * P:(ot + 1) * P, qq * QW:(qq + 1) * QW]
        ob = ob_pool.tile([P, QW], bf16, tag="ob")
        if qq == NQ - 1 and ot % 2 == 1:
            nc.scalar.copy(ob[:], ps[:])   # tail: ACT is idle by then
        else:
            nc.vector.tensor_copy(ob[:], ps[:])
        nc.sync.dma_start(dst, ob[:])

    def run_opq(n):
        while n > 0 and opq:
            kind, h_or_ot, qq = opq.pop(0)
            if kind == "tr":
                transp_unit(h_or_ot, qq)
            else:
                op_unit(qq, h_or_ot)
            n -= 1

    # ---- session machinery: scores/exp/mask for (h, qq); the PREVIOUS
    # session's ctx matmuls and normalization weave into this session's slots
    # so the exp stream never waits on ctx inputs.
    def ctx_qb(st, qb):
        # one q-block's FULL 16-step accumulation as a consecutive run: a
        # PSUM bank supports only one open accumulation group at a time, so
        # groups sharing the cx bank must never interleave
        h = st["h"]
        for tt in range(KS):
            require(("v", h // 2, tt))
        for ks in range(KS):
            pt = st["pts"][ks // 2]
            j = ks % 2
            nc.tensor.matmul(
                st["cxv"][:, qb, 0:VW],
                pt[:, j * QW + qb * P: j * QW + (qb + 1) * P],
                v_view[:, ks, h, :],
                start=(ks == 0),
                stop=(ks == KS - 1),
            )

    tr_open = [False]
    tr_pushed = set()

    def norm(st):
        h, qq, cxv = st["h"], st["qq"], st["cxv"]
        blocks = []
        for qb in range(4):
            rs = rs_pool.tile([P, 1], f32, tag="rs")
            nc.vector.reciprocal(rs[:], cxv[:, qb, DH:DH + 1])
            cn_t = cn_pool.tile([P, DH], bf16, tag="cn", name=f"cn{h}_{qq}_{qb}")
            nc.vector.tensor_scalar_mul(cn_t[:], cxv[:, qb, 0:DH], rs[:])
            if dbg is not None and h == 0:
                nc.sync.dma_start(
                    dbg["cn"].ap()[(qq * 4 + qb) * P:(qq * 4 + qb + 1) * P, :], cn_t[:]
                )
            blocks.append(cn_t)
        cn_tiles[(h, qq)] = blocks
        if tr_open[0]:
            # queue transposes for every normalized head of this quarter
            for hh in range(4):
                if (hh, qq) in cn_tiles and (hh, qq) not in tr_pushed:
                    tr_pushed.add((hh, qq))
                    opq.append(("tr", hh, qq))
        if h == 3:
            for ot in range(8):
                opq.append(("op", ot, qq))

    def run_session(h, qq, prev, quota, ctx_start=0, ctx_end=6, self_ctx=False):
        ft, ro = h // 2, (h % 2) * DH
        require(("q", ft, qq, 1))
        # 4 q-block accumulation regions, each 512B-aligned within one bank
        cx = cx_pool.tile([P, 4 * P], f32, tag="cx")
        st = {
            "h": h, "qq": qq, "pts": [None] * NPAIR,
            "cxv": cx[:].rearrange("p (b w) -> p b w", b=4),
        }
        nmv = nm_tiles[qq][:].rearrange("p (ks q) -> p ks q", ks=KS)
        ctx_slots = [[] for _ in range(NPAIR)]
        if prev is not None:
            nsl = ctx_end - ctx_start
            for qb in range(4):
                ctx_slots[ctx_start + (qb * nsl) // 4].append(qb)
        pumped = 0
        for p in range(NPAIR):
            require(("k", ft, p // 2, 1))
            ps = sc_pool.tile([P, 2 * QW], f32, tag="sc")
            for j in (0, 1):
                ks = 2 * p + j
                nc.tensor.matmul(
                    ps[:, j * QW:(j + 1) * QW],
                    kT[ft][ro:ro + DH, ks * P:(ks + 1) * P],
                    qT[ft][ro:ro + DH, qq * QW:(qq + 1) * QW],
                    start=True,
                    stop=True,
                )
            pt = pt_pool.tile([P, 2 * QW], bf16, tag="pt")
            nc.scalar.activation(pt[:], ps[:], Exp, scale=0.125)
            ptv = pt[:].rearrange("p (j q) -> p j q", j=2)
            nc.vector.tensor_mul(ptv, ptv, nmv[:, 2 * p:2 * p + 2, :])
            st["pts"][p] = pt
            if dbg is not None and h == 0:
                nc.sync.dma_start(
                    dbg["pT0"].ap().rearrange(
                        "(pp two pr) q -> pr pp two q", pr=P, two=2)
                    [:, p, :, qq * QW:(qq + 1) * QW],
                    pt[:].rearrange("p (two q) -> p two q", two=2),
                )
            if prev is not None:
                # keep the v-projection weave paced even before its q-block
                # chunk needs it
                require(("v", prev["h"] // 2, 2 * p))
                require(("v", prev["h"] // 2, 2 * p + 1))
                for qb in ctx_slots[p]:
                    ctx_qb(prev, qb)
                if p == ctx_end:
                    norm(prev)
            if opq:
                run_opq(2)
            elif pumped < quota:
                pump(1)
                pumped += 1
        if prev is not None and ctx_end >= NPAIR:
            norm(prev)
        if self_ctx:
            for qb in range(4):
                ctx_qb(st, qb)
            norm(st)
        return st

    # ---- main schedule: 16 sessions, software-pipelined ----
    order = [(h0 + dh, qq) for ph, h0 in ((1, 0), (2, 2)) for qq in range(NQ)
             for dh in (0, 1)]
    prev = None
    op_cm = tr_cm = None
    for si, (h, qq) in enumerate(order):
        # mask prefetch for phase 2 (phase-1 masks are all in the prologue)
        if (h, qq) == (1, 2):
            load_nm(1, 0)
        elif (h, qq) == (3, 0):
            load_nm(1, 1)
        elif (h, qq) == (3, 1):
            load_nm(1, 2)
        elif (h, qq) == (3, 2):
            load_nm(1, 3)
        if si == 9:
            # all projection work must be done now; swap pj's PSUM banks to
            # the transpose + output-projection pools
            while foreign:
                pump(1)
            pj_cm.__exit__(None, None, None)
            tr_cm = tc.tile_pool(name="tr", bufs=1, space="PSUM")
            tr_pool = tr_cm.__enter__()
            op_cm = tc.tile_pool(name="op", bufs=1, space="PSUM")
            op_pool = op_cm.__enter__()
            tr_open[0] = True
        prev = run_session(h, qq, prev,
                           quota=0 if si <= 1 else (8 if si == 8 else 2),
                           ctx_start=6 if si == 1 else 2,
                           ctx_end=8 if si == 1 else 7,
                           self_ctx=(si == 15))

    # tail: last quarter's transposes, then a double-buffered outproj burst
    while opq and opq[0][0] == "tr":
        kind, hh, qq = opq.pop(0)
        transp_unit(hh, qq)
    op_cm.__exit__(None, None, None)
    tr_cm.__exit__(None, None, None)
    cx_cm.__exit__(None, None, None)
    op_cm = tc.tile_pool(name="op2", bufs=4, space="PSUM")
    op_pool = op_cm.__enter__()
    run_opq(len(opq))

    if dbg is not None:
        for ft in range(2):
            nc.sync.dma_start(dbg["qT"].ap()[ft * P:(ft + 1) * P, :], qT[ft][:])
            nc.sync.dma_start(dbg["kT"].ap()[ft * P:(ft + 1) * P, :], kT[ft][:])
        nc.sync.dma_start(dbg["v"].ap(), v_sb[:])
        for qq in range(NQ):
            for fs in range(2):
                nc.sync.dma_start(
                    dbg["ctxT"].ap()[fs * P:(fs + 1) * P, qq * QW:(qq + 1) * QW],
                    ctxT[(qq, fs)][:],
                )

    assert not foreign and not opq, f"undrained work: {foreign} {opq}"
    op_cm.__exit__(None, None, None)
    es.close()


@functools.lru_cache(maxsize=1)
def _build(debug=False):
    import concourse.bacc as bacc
    import concourse.mybir as mybir
    import concourse.tile as tile

    bf16 = mybir.dt.bfloat16
    f32 = mybir.dt.float32

    nc = bacc.Bacc("TRN2", target_bir_lowering=False, debug=False, num_devices=NCORES)
    t = {
        "xqT": nc.dram_tensor("xqT", (D, S), bf16, kind="ExternalInput"),
        "xkT": nc.dram_tensor("xkT", (D, S), bf16, kind="ExternalInput"),
        "xvT": nc.dram_tensor("xvT", (D, S), bf16, kind="ExternalInput"),
        "wqT": nc.dram_tensor("wqT", (D, F), bf16, kind="ExternalInput"),
        "wkT": nc.dram_tensor("wkT", (D, F), bf16, kind="ExternalInput"),
        "wvT": nc.dram_tensor("wvT", (D, F), bf16, kind="ExternalInput"),
        "womT": nc.dram_tensor("womT", (F, D), bf16, kind="ExternalInput"),
        "nmT": nc.dram_tensor("nmT", (S, S), bf16, kind="ExternalInput"),
        "bq": nc.dram_tensor("bq", (F, 1), f32, kind="ExternalInput"),
        "bk": nc.dram_tensor("bk", (F, 1), f32, kind="ExternalInput"),
        "ident": nc.dram_tensor("ident", (P, P), bf16, kind="ExternalInput"),
        "outT": nc.dram_tensor("outT", (D, S), bf16, kind="ExternalOutput"),
    }
    dbg = None
    if debug:
        dbg = {
            "qT": nc.dram_tensor("dbg_qT", (F, S), bf16, kind="ExternalOutput"),
            "kT": nc.dram_tensor("dbg_kT", (F, S), bf16, kind="ExternalOutput"),
            "v": nc.dram_tensor("dbg_v", (P, KS * NH * VW), bf16, kind="ExternalOutput"),
            "pT0": nc.dram_tensor("dbg_pT0", (S, S), bf16, kind="ExternalOutput"),
            "ctxT": nc.dram_tensor("dbg_ctxT", (F, S), bf16, kind="ExternalOutput"),
            "cn": nc.dram_tensor("dbg_cn", (S, DH), bf16, kind="ExternalOutput"),
        }
    with tile.TileContext(nc) as tc:
        _emit(nc, tc, t, dbg)
    nc.compile()
    return nc


def _prep_core_inputs(c, Q, K, V, mask, Wq, bq, Wk, bk, Wv, Wo, _cache={}):
    import ml_dtypes

    bf = ml_dtypes.bfloat16
    b, g = divmod(c, GROUPS)
    bkey = ("batch", b, id(Q))
    if bkey not in _cache:
        _cache.clear()
        for bb in range(B):
            nm = 1.0 - mask[bb, 0].astype(np.float32)
            _cache[("batch", bb, id(Q))] = {
                "xqT": Q[bb].T.astype(bf),
                "xkT": K[bb].T.astype(bf),
                "xvT": V[bb].T.astype(bf),
                "nmT": nm.T.astype(bf),
            }
    fsl = slice(g * F, (g + 1) * F)
    return {
        **_cache[bkey],
        "wqT": Wq[fsl, :].T.astype(bf),
        "wkT": Wk[fsl, :].T.astype(bf),
        "wvT": Wv[fsl, :].T.astype(bf),
        "womT": Wo[:, fsl].T.astype(bf),
        "bq": bq[fsl].reshape(F, 1).astype(np.float32),
        "bk": bk[fsl].reshape(F, 1).astype(np.float32),
        "ident": np.eye(P, dtype=bf),
    }


def kernel(Q, K, V, mask, Wq, bq, Wk, bk, Wv, bv, Wo, bo, _trace=False, _tmpdir=None):
    from concourse.bass_utils import run_bass_kernel_spmd

    Q, K, V = np.asarray(Q, np.float32), np.asarray(K, np.float32), np.asarray(V, np.float32)
    mask = np.asarray(mask)
    Wq, Wk, Wv, Wo = (np.asarray(w, np.float32) for w in (Wq, Wk, Wv, Wo))
    bq, bk, bv, bo = (np.asarray(x, np.float32) for x in (bq, bk, bv, bo))

    nc = _build()
    in_maps = [_prep_core_inputs(c, Q, K, V, mask, Wq, bq, Wk, bk, Wv, Wo) for c in range(NCORES)]
    kw = {}
    if _trace:
        kw = dict(trace=True, tmpdir=_tmpdir)
    res = run_bass_kernel_spmd(nc, in_maps, core_ids=list(range(NCORES)), **kw)

    const = (Wo @ bv + bo).astype(np.float32)  # softmax rows sum to 1 -> bv enters linearly
    out = np.empty((B, S, D), np.float32)
    for b in range(B):
        acc = res.results[b * GROUPS]["outT"].astype(np.float32)
        for g in range(1, GROUPS):
            acc = acc + res.results[b * GROUPS + g]["outT"].astype(np.float32)
        out[b] = acc.T + const
    if _trace:
        kernel._last_results = res
    return out
